# revision 2
# baseline (speedup 1.0000x reference)
"""BottleneckAttn TRN2 kernel.

Reference computation (per batch b, head n, fp32):
    qkv = w_qkv @ x_b                      # (1536, 1024), 1x1 conv
    q, k, v per head: (1024, 128) with hw = h*32 + w
    logits[q,k] = SCALE * (q . k) + qw[q, 31 + w2(k) - w(q)] + qh[q, 31 + h2(k) - h(q)]
        where qw[q,r] = q . width_rel[r], qh[q,r] = q . height_rel[r]
    out = softmax(logits) @ v              # (1024, 128)
    output[b] flat index = q*512 + n*128 + d  -> reshape (512, 32, 32)

Device strategy (SPMD, 8 cores, 2 batches/core):
  - All matmuls computed in the TRANSPOSED softmax layout ST[k, q] so the
    attention probabilities come out of the exp directly in the layout the
    PV matmul needs as its moving operand (no P transposes).
  - ST = k @ qT via PE (operands swapped); the relative-position bias is
    folded in as a second accumulating matmul with a constant 0/1 selection
    matrix lhsT (rows select the shifted width/height tables per PSUM
    partition).
  - The per-query shifted tables (skew gather) are built by a DRAM round
    trip: qw/qh computed in [q, r] layout on PE, stored to a DRAM scratch,
    re-loaded with an affine skewed access pattern (contiguous 32-element
    runs), then rotated into [table_row, q] layout with DVE 32x32 stream
    transposes.
  - Softmax denominators via an all-ones stationary matmul accumulated over
    k tiles into one PSUM bank (partitions 0/32 for the two q halves);
    output normalized on the host at (d x q) scale, not (k x q).
  - All SBUF operands are 16-bit (bf16 compute / fp16 bias tables): same PE
    column rate as float32r but half the DMA and SBUF traffic.
  - x is pre-swizzled to the device's f-order hw columns on the host, so
    the projection matmuls read plain contiguous slices.
  - den/PV matmuls for k-tile kc are emitted after the ST+bias matmuls of
    kc+1, so the scalar-engine exp has a full k-tile of slack to hide in.
"""

import os
import sys

import numpy as np

for _p in ("/opt/trn_rl_repo", "/root/.axon_site/_ro/trn_rl_repo"):
    if os.path.isdir(_p) and _p not in sys.path:
        sys.path.append(_p)

import ml_dtypes

import concourse.bass as bass
import concourse.mybir as mybir
import concourse.tile as tile
from concourse import bacc
from concourse.bass_utils import run_bass_kernel_spmd

B, C, H, W = 16, 512, 32, 32
HW = H * W
NH, DH = 4, 128
SCALE = DH ** -0.5
N_CORES = 8
B_LOC = B // N_CORES

F32 = mybir.dt.float32
BF16 = mybir.dt.bfloat16
FP16 = mybir.dt.float16
EXP = mybir.ActivationFunctionType.Exp

_CACHE = {}

# f-order permutation: device hw column f = 256*s + 32*qt + j for q = 128*qt + 32*s + j
_QS = np.arange(1024)
_F_OF_Q = (256 * ((_QS % 128) // 32) + 32 * (_QS // 128) + (_QS % 32)).astype(np.int64)
_Q_OF_F = np.argsort(_F_OF_Q)


def _sel_const():
    """sel[j, kc*128 + p]: j<32 selects shifted width row w2(p), j in [32,64)
    selects shifted height row h2(p); rows 64..127 are zero padding."""
    sel = np.zeros((128, 8 * 128), np.float32)
    for kc in range(8):
        for p in range(128):
            f = 128 * kc + p
            sg, qt, j = f // 256, (f % 256) // 32, f % 32
            q = 128 * qt + 32 * sg + j
            sel[q % 32, kc * 128 + p] = 1.0
            sel[32 + q // 32, kc * 128 + p] = 1.0
    sel[64:128, :] = sel[0:64, :]  # duplicate for the upper-PE-half variant
    return sel  # [128, 1024]


def _emit(tc, nc, xd, wd, reld, seld, onesd, identd, outd, dend, scr_handles):
    from contextlib import ExitStack

    ctx = ExitStack()
    with ctx:
        const = ctx.enter_context(tc.tile_pool(name="const", bufs=1))
        xpool = ctx.enter_context(tc.tile_pool(name="x", bufs=B_LOC))
        qkvp = ctx.enter_context(tc.tile_pool(name="qkv", bufs=3))
        qwhp = ctx.enter_context(tc.tile_pool(name="qwh", bufs=3))
        wvp = ctx.enter_context(tc.tile_pool(name="wv", bufs=3))
        biasp = ctx.enter_context(tc.tile_pool(name="biasv", bufs=3))
        vnatp = ctx.enter_context(tc.tile_pool(name="vnat", bufs=2))
        ptp = ctx.enter_context(tc.tile_pool(name="pt", bufs=3))
        outp = ctx.enter_context(tc.tile_pool(name="outt", bufs=2))
        recp = ctx.enter_context(tc.tile_pool(name="recip", bufs=2))
        # PSUM budget (8 banks): st 3 + out 2 + den 1 + misc 2
        ps_st = ctx.enter_context(tc.tile_pool(name="psst", bufs=3, space="PSUM"))
        ps_out = ctx.enter_context(tc.tile_pool(name="psout", bufs=2, space="PSUM"))
        ps_den = ctx.enter_context(tc.tile_pool(name="psden", bufs=1, space="PSUM"))
        ps_misc = ctx.enter_context(tc.tile_pool(name="psmisc", bufs=2, space="PSUM"))

        # ---- constants / weights ----
        x_sb = []

        def _load_x(b):
            xb = xpool.tile([128, 4 * HW], BF16, tag="x", name=f"x_sb{b}")
            for kc4 in range(4):
                for qc in range(2):
                    xeng = nc.scalar if (kc4 * 2 + qc) % 2 == 0 else nc.sync
                    xeng.dma_start(
                        xb[:, kc4 * HW + qc * 512 : kc4 * HW + qc * 512 + 512],
                        xd[b][kc4 * 128 : kc4 * 128 + 128, qc * 512 : qc * 512 + 512],
                    )
            x_sb.append(xb)

        _load_x(0)
        wt_sb = const.tile([128, 4 * 1536], BF16, name="wt_sb")
        # h3-outer: all q weights land first, then k, then v; alternate queues
        for h3 in range(3):
            for kc4 in range(4):
                weng = nc.sync if (h3 * 4 + kc4) % 2 == 0 else nc.scalar
                weng.dma_start(
                    wt_sb[:, kc4 * 1536 + h3 * 512 : kc4 * 1536 + h3 * 512 + 512],
                    wd[kc4 * 128 : kc4 * 128 + 128, h3 * 512 : h3 * 512 + 512],
                )
        rel_sb = const.tile([128, 256], BF16, name="rel_sb")
        nc.gpsimd.dma_start(rel_sb[:], reld)
        sel_sb = const.tile([128, 1024], FP16, name="sel_sb")
        nc.gpsimd.dma_start(sel_sb[:], seld)
        ones_sb = const.tile([128, 1], BF16, name="ones_sb")
        nc.gpsimd.dma_start(ones_sb[:], onesd)
        id_sb = const.tile([128, 128], BF16, name="id_sb")
        nc.gpsimd.dma_start(id_sb[:], identd)

        def _proj(bn, qkvT, t, col0):
            b = bn // NH
            for qc in range(2):
                ps = ps_misc.tile(
                    [128, 512], F32, tag="misc", name=f"proj{bn}_{t}_{qc}"
                )
                for kc4 in range(4):
                    nc.tensor.matmul(
                        ps[:],
                        wt_sb[:, kc4 * 1536 + col0 : kc4 * 1536 + col0 + 128],
                        x_sb[b][:, kc4 * HW + qc * 512 : kc4 * HW + qc * 512 + 512],
                        start=(kc4 == 0),
                        stop=(kc4 == 3),
                    )
                nc.vector.tensor_copy(
                    qkvT[:, t * HW + qc * 512 : t * HW + qc * 512 + 512], ps[:]
                )

        def stage_bias(bn):
            """q projection + rel tables + skew round trip -> (qkvT, bias_vecT).

            Emitted one bn ahead so the DRAM round trip and the DVE stream
            transposes hide under the previous bn's attention matmuls."""
            n = bn % NH
            if bn == NH:
                _load_x(1)
            scr = scr_handles[bn]
            qkvT = qkvp.tile([128, 3 * HW], BF16, tag="qkv", name=f"qkvT{bn}")
            _proj(bn, qkvT, 0, n * DH)  # q only

            qwh = qwhp.tile([128, 8 * 126], FP16, tag="qwh", name=f"qwh{bn}")
            for qt in range(8):
                ps = ps_misc.tile([128, 256], F32, tag="misc", name=f"qwhp{bn}_{qt}")
                nc.tensor.matmul(
                    ps[:],
                    qkvT[:, qt * 128 : qt * 128 + 128],
                    rel_sb[:],
                    start=True,
                    stop=True,
                )
                nc.scalar.copy(qwh[:, qt * 126 : qt * 126 + 126], ps[:, :126])

            nc.sync.dma_start(
                scr.ap().rearrange("(a p) r -> p a r", p=128),
                qwh[:].rearrange("p (a r) -> p a r", r=126),
            )
            wv = wvp.tile([128, 256], FP16, tag="wv", name=f"wv{bn}")
            hv = wvp.tile([128, 256], FP16, tag="hv", name=f"hv{bn}")
            skew_eng = nc.sync if bn < 2 else nc.gpsimd
            for sg in range(4):
                skew_eng.dma_start(
                    wv[32 * sg : 32 * sg + 32, :].rearrange("p (a j) -> p a j", j=32),
                    bass.AP(scr, 31 + 32256 * sg, [[125, 32], [4032, 8], [1, 32]]),
                )
                skew_eng.dma_start(
                    hv[32 * sg : 32 * sg + 32, :].rearrange("p (a j) -> p a j", j=32),
                    bass.AP(scr, 94 + 32255 * sg, [[126, 32], [4028, 8], [1, 32]]),
                )

            bias_vecT = biasp.tile([128, HW], FP16, tag="biasv", name=f"biasv{bn}")
            for src, row0 in ((wv, 0), (hv, 32)):
                for sg in range(4):
                    nc.vector.transpose(
                        bias_vecT[row0 : row0 + 32, 256 * sg : 256 * sg + 256],
                        src[32 * sg : 32 * sg + 32, :],
                    )
            nc.vector.tensor_copy(bias_vecT[64:128, :], bias_vecT[0:64, :])
            return qkvT, bias_vecT

        def stage_kv(bn, qkvT):
            n = bn % NH
            _proj(bn, qkvT, 1, 512 + n * DH)  # k
            _proj(bn, qkvT, 2, 1024 + n * DH)  # v
            vnat = vnatp.tile([128, HW], BF16, tag="vnat", name=f"vnat{bn}")
            for kc in range(8):
                ps = ps_misc.tile([128, 128], BF16, tag="misc", name=f"vtr{bn}_{kc}")
                nc.tensor.transpose(
                    ps[:], qkvT[:, 2 * HW + kc * 128 : 2 * HW + kc * 128 + 128], id_sb[:]
                )
                nc.vector.tensor_copy(vnat[:, kc * 128 : kc * 128 + 128], ps[:])
            return vnat

        def stage_attn(bn, qkvT, bias_vecT, vnat):
            b, n = bn // NH, bn % NH

            den_ps = ps_den.tile([64, 512], F32, tag="den", name=f"den{bn}")
            out_ps = [
                ps_out.tile([128, 512], F32, tag="out", name=f"outp{bn}_{i}")
                for i in range(2)
            ]

            def emit_st(kc):
                pT = ptp.tile([128, HW], BF16, tag="pt", name=f"pt{bn}_{kc}")
                for qc in range(2):
                    st = ps_st.tile([128, 512], F32, tag="st", name=f"st{bn}_{kc}_{qc}")
                    nc.tensor.matmul(
                        st[:],
                        qkvT[:, HW + kc * 128 : HW + kc * 128 + 128],
                        qkvT[:, qc * 512 : qc * 512 + 512],
                        start=True,
                        stop=False,
                    )
                    r0 = 64 * qc
                    nc.tensor.matmul(
                        st[:],
                        sel_sb[r0 : r0 + 64, kc * 128 : kc * 128 + 128],
                        bias_vecT[r0 : r0 + 64, qc * 512 : qc * 512 + 512],
                        start=False,
                        stop=True,
                    )
                    nc.scalar.activation(
                        pT[:, qc * 512 : qc * 512 + 512], st[:], EXP
                    )
                return pT

            def emit_dp(kc, pT):
                for qc in range(2):
                    nc.tensor.matmul(
                        den_ps[32 * qc : 32 * qc + 1, :],
                        ones_sb[:],
                        pT[:, qc * 512 : qc * 512 + 512],
                        start=(kc == 0),
                        stop=(kc == 7),
                    )
                    nc.tensor.matmul(
                        out_ps[qc][:],
                        vnat[:, kc * 128 : kc * 128 + 128],
                        pT[:, qc * 512 : qc * 512 + 512],
                        start=(kc == 0),
                        stop=(kc == 7),
                    )

            prev = None  # one kc behind: den/pv hide the exp latency
            for kc in range(8):
                pT = emit_st(kc)
                if prev is not None:
                    emit_dp(kc - 1, prev)
                prev = pT
            emit_dp(7, prev)

            outT = outp.tile([128, HW], F32, tag="outt", name=f"outT{bn}")
            den_sb = recp.tile([1, HW], F32, tag="densb", name=f"densb{bn}")
            for qc in range(2):
                nc.vector.tensor_copy(
                    outT[:, qc * 512 : qc * 512 + 512], out_ps[qc][:]
                )
                nc.scalar.copy(
                    den_sb[:, qc * 512 : qc * 512 + 512],
                    den_ps[32 * qc : 32 * qc + 1, :],
                )
            nc.sync.dma_start(outd[b, n], outT[:])
            nc.sync.dma_start(dend[b, n], den_sb[:])

        # software pipeline: bias chains emitted two bn ahead of attention
        n_bn = B_LOC * NH
        from collections import deque

        states = deque([stage_bias(0)])
        kvs = deque([stage_kv(0, states[0][0])])
        states.append(stage_bias(1))
        for bn in range(n_bn):
            if bn + 1 < n_bn:
                kvs.append(stage_kv(bn + 1, states[1][0]))
            if bn + 2 < n_bn:
                states.append(stage_bias(bn + 2))
            qkvT, bias_vecT = states.popleft()
            stage_attn(bn, qkvT, bias_vecT, kvs.popleft())


def _build():
    if "nc" in _CACHE:
        return _CACHE["nc"]
    nc = bacc.Bacc("TRN2", target_bir_lowering=False, debug=False, num_devices=N_CORES)
    xd = nc.dram_tensor("x_r", [B_LOC, C, HW], BF16, kind="ExternalInput").ap()
    wd = nc.dram_tensor("w_t", [C, 3 * NH * DH], BF16, kind="ExternalInput").ap()
    reld = nc.dram_tensor("rel_t", [128, 256], BF16, kind="ExternalInput").ap()
    seld = nc.dram_tensor("sel", [128, 1024], FP16, kind="ExternalInput").ap()
    onesd = nc.dram_tensor("ones", [128, 1], BF16, kind="ExternalInput").ap()
    identd = nc.dram_tensor("ident", [128, 128], BF16, kind="ExternalInput").ap()
    outd = nc.dram_tensor("out_r", [B_LOC, NH, DH, HW], F32, kind="ExternalOutput").ap()
    dend = nc.dram_tensor("den_r", [B_LOC, NH, 1, HW], F32, kind="ExternalOutput").ap()
    scr_handles = [
        nc.dram_tensor(f"scr{i}", [HW, 126], FP16) for i in range(B_LOC * NH)
    ]
    with tile.TileContext(nc) as tc:
        _emit(tc, nc, xd, wd, reld, seld, onesd, identd, outd, dend, scr_handles)
    nc.compile()
    _CACHE["nc"] = nc
    return nc


def _in_maps(x, w_qkv, height_rel, width_rel):
    x = np.asarray(x, np.float32)
    w_qkv = np.asarray(w_qkv, np.float32)
    height_rel = np.asarray(height_rel, np.float32)
    width_rel = np.asarray(width_rel, np.float32)

    w_t = np.ascontiguousarray(w_qkv.T)  # [C, 1536]
    w_t[:, 512:1024] *= np.float32(SCALE)  # fold softmax scale into k
    w_t = w_t.astype(ml_dtypes.bfloat16)
    rel_t = np.zeros((128, 256), np.float32)
    rel_t[:, 0:63] = width_rel.T
    rel_t[:, 63:126] = height_rel.T
    rel_t = rel_t.astype(ml_dtypes.bfloat16)
    sel = _sel_const().astype(np.float16)
    ones = np.ones((128, 1), np.float32).astype(ml_dtypes.bfloat16)
    ident = np.eye(128, dtype=np.float32).astype(ml_dtypes.bfloat16)

    # pre-swizzle x's hw columns into device f-order
    xf = x.reshape(B, C, HW)[:, :, _Q_OF_F].astype(ml_dtypes.bfloat16)

    shared = {
        "w_t": w_t,
        "rel_t": rel_t,
        "sel": sel,
        "ones": ones,
        "ident": ident,
    }
    maps = []
    for i in range(N_CORES):
        xm = xf[i * B_LOC : (i + 1) * B_LOC]
        maps.append({"x_r": np.ascontiguousarray(xm), **shared})
    return maps


def _assemble(results):
    out = np.empty((B, 3 * NH * DH // 3, H, W), np.float32)  # (16, 512, 32, 32)
    for i, r in enumerate(results):
        arr = r["out_r"] / r["den_r"]  # [B_LOC, NH, DH, HW] / [B_LOC, NH, 1, HW]
        arr = arr[..., _F_OF_Q]  # undo the device-side f-ordering of hw columns
        for b in range(B_LOC):
            # flat order of reference output = q*512 + n*128 + d
            out[i * B_LOC + b] = (
                arr[b].transpose(2, 0, 1).reshape(512, 32, 32)
            )
    return out


def run(x, w_qkv, height_rel, width_rel, **spmd_kwargs):
    nc = _build()
    maps = _in_maps(x, w_qkv, height_rel, width_rel)
    res = run_bass_kernel_spmd(nc, maps, core_ids=list(range(N_CORES)), **spmd_kwargs)
    return _assemble(res.results), res


def kernel(x, w_qkv, height_rel, width_rel):
    out, _ = run(x, w_qkv, height_rel, width_rel)
    return out


# revision 14
# speedup vs baseline: 1.0810x; 1.0810x over previous
"""BottleneckAttn TRN2 kernel.

Reference computation (per batch b, head n, fp32):
    qkv = w_qkv @ x_b                      # (1536, 1024), 1x1 conv
    q, k, v per head: (1024, 128) with hw = h*32 + w
    logits[q,k] = SCALE * (q . k) + qw[q, 31 + w2(k) - w(q)] + qh[q, 31 + h2(k) - h(q)]
        where qw[q,r] = q . width_rel[r], qh[q,r] = q . height_rel[r]
    out = softmax(logits) @ v              # (1024, 128)
    output[b] flat index = q*512 + n*128 + d  -> reshape (512, 32, 32)

Device strategy (SPMD, 8 cores, 2 batches/core):
  - All matmuls computed in the TRANSPOSED softmax layout ST[k, q] so the
    attention probabilities come out of the exp directly in the layout the
    PV matmul needs as its moving operand (no P transposes).
  - ST = k @ qT via PE (operands swapped); the relative-position bias is
    folded in as a second accumulating matmul with a constant 0/1 selection
    matrix lhsT (rows select the shifted width/height tables per PSUM
    partition).
  - The per-query shifted tables (skew gather) are built by a DRAM round
    trip: qw/qh computed in [q, r] layout on PE, stored to a DRAM scratch,
    re-loaded with an affine skewed access pattern (contiguous 32-element
    runs), then rotated into [table_row, q] layout with DVE 32x32 stream
    transposes.
  - Softmax denominators via an all-ones stationary matmul accumulated over
    k tiles into a single PSUM bank (partitions 0/32 for the two q halves).
  - EVERY matmul runs float32r (fp32_mode=HIGH): measured fastest issue
    rate (~237ns per 512-col matmul, overlapped); 16-bit operand modes
    measured SLOWER (~260-390ns) and mode switches cost ~100ns, so the
    sel/bias chain is f32r too (baseline used fp16 there).
  - den/PV matmuls for k-tile kc are emitted after the ST+bias matmuls of
    kc+1 so the scalar-engine exp has a full k-tile of slack to hide in.
  - x is pre-swizzled to the device's f-order hw columns on the host, so
    the projection matmuls read plain contiguous slices.
"""

import os
import sys

import numpy as np

for _p in ("/opt/trn_rl_repo", "/root/.axon_site/_ro/trn_rl_repo"):
    if os.path.isdir(_p) and _p not in sys.path:
        sys.path.append(_p)

import concourse.bass as bass
import concourse.mybir as mybir
import concourse.tile as tile
from concourse import bacc
from concourse.bass_utils import run_bass_kernel_spmd

B, C, H, W = 16, 512, 32, 32
HW = H * W
NH, DH = 4, 128
SCALE = DH ** -0.5
N_CORES = 8
B_LOC = B // N_CORES

F32 = mybir.dt.float32
F32R = mybir.dt.float32r
EXP = mybir.ActivationFunctionType.Exp

_CACHE = {}

# f-order permutation: device hw column f = 256*s + 32*qt + j for q = 128*qt + 32*s + j
_QS = np.arange(1024)
_F_OF_Q = (256 * ((_QS % 128) // 32) + 32 * (_QS // 128) + (_QS % 32)).astype(np.int64)
_Q_OF_F = np.argsort(_F_OF_Q)


def _sel_const():
    """sel[j, kc*128 + p]: j<32 selects shifted width row w2(p),
    j in [32,64) selects shifted height row h2(p); both PE row halves."""
    sel = np.zeros((64, 8 * 128), np.float32)
    for kc in range(8):
        for p in range(128):
            f = 128 * kc + p
            sg, qt, j = f // 256, (f % 256) // 32, f % 32
            q = 128 * qt + 32 * sg + j
            sel[q % 32, kc * 128 + p] = 1.0
            sel[32 + q // 32, kc * 128 + p] = 1.0
    return np.concatenate([sel, sel], axis=0)  # [128, 1024]


def _emit(tc, nc, xd, wd, reld, seld, onesd, identd, outd, dend, scr_handles):
    from contextlib import ExitStack

    ctx = ExitStack()
    with ctx:
        const = ctx.enter_context(tc.tile_pool(name="const", bufs=1))
        xpool = ctx.enter_context(tc.tile_pool(name="x", bufs=B_LOC))
        qkvp = ctx.enter_context(tc.tile_pool(name="qkv", bufs=3))
        qwhp = ctx.enter_context(tc.tile_pool(name="qwh", bufs=3))
        wvp = ctx.enter_context(tc.tile_pool(name="wv", bufs=3))
        biasp = ctx.enter_context(tc.tile_pool(name="biasv", bufs=3))
        vnatp = ctx.enter_context(tc.tile_pool(name="vnat", bufs=2))
        ptp = ctx.enter_context(tc.tile_pool(name="pt", bufs=3))
        outp = ctx.enter_context(tc.tile_pool(name="outt", bufs=2))
        recp = ctx.enter_context(tc.tile_pool(name="recip", bufs=2))
        # PSUM budget (8 banks): st 2 + out 2 + den 2 + misc 2
        ps_st = ctx.enter_context(tc.tile_pool(name="psst", bufs=2, space="PSUM"))
        ps_out = ctx.enter_context(tc.tile_pool(name="psout", bufs=2, space="PSUM"))
        ps_den = ctx.enter_context(tc.tile_pool(name="psden", bufs=2, space="PSUM"))
        ps_misc = ctx.enter_context(tc.tile_pool(name="psmisc", bufs=2, space="PSUM"))

        # ---- constants / weights (spread startup DMA over 4 queues) ----
        x_sb = []
        qs = [nc.scalar, nc.sync, nc.gpsimd]

        def _load_x(b):
            xb = xpool.tile([128, 4 * HW], F32R, tag="x", name=f"x_sb{b}")
            for kc4 in range(4):
                for qc in range(2):
                    xeng = qs[(kc4 * 2 + qc) % 3]
                    xeng.dma_start(
                        xb[:, kc4 * HW + qc * 512 : kc4 * HW + qc * 512 + 512],
                        xd[b][kc4 * 128 : kc4 * 128 + 128, qc * 512 : qc * 512 + 512],
                    )
            x_sb.append(xb)

        _load_x(0)
        wt_sb = const.tile([128, 4 * 1536], F32R, name="wt_sb")
        # h3-outer: all q weights land first, then k, then v; alternate queues
        for h3 in range(3):
            for kc4 in range(4):
                weng = qs[(h3 * 4 + kc4 + 1) % 3]
                weng.dma_start(
                    wt_sb[:, kc4 * 1536 + h3 * 512 : kc4 * 1536 + h3 * 512 + 512],
                    wd[kc4 * 128 : kc4 * 128 + 128, h3 * 512 : h3 * 512 + 512],
                )
        rel_sb = const.tile([128, 256], F32R, name="rel_sb")
        nc.gpsimd.dma_start(rel_sb[:], reld)
        sel_sb = const.tile([128, 1024], F32R, name="sel_sb")
        nc.gpsimd.dma_start(sel_sb[:], seld)
        ones_sb = const.tile([128, 1], F32R, name="ones_sb")
        nc.gpsimd.dma_start(ones_sb[:], onesd)
        id_sb = const.tile([128, 128], F32R, name="id_sb")
        nc.gpsimd.dma_start(id_sb[:], identd)

        def _proj(bn, qkvT, t, col0):
            b = bn // NH
            for qc in range(2):
                ps = ps_misc.tile(
                    [128, 512], F32, tag="misc", name=f"proj{bn}_{t}_{qc}"
                )
                for kc4 in range(4):
                    nc.tensor.matmul(
                        ps[:],
                        wt_sb[:, kc4 * 1536 + col0 : kc4 * 1536 + col0 + 128],
                        x_sb[b][:, kc4 * HW + qc * 512 : kc4 * HW + qc * 512 + 512],
                        start=(kc4 == 0),
                        stop=(kc4 == 3),
                    )
                if (t + qc) % 2 == 0:
                    nc.vector.tensor_copy(
                        qkvT[:, t * HW + qc * 512 : t * HW + qc * 512 + 512], ps[:]
                    )
                else:
                    nc.scalar.copy(
                        qkvT[:, t * HW + qc * 512 : t * HW + qc * 512 + 512], ps[:]
                    )

        def stage_bias(bn):
            """q projection + rel tables + skew round trip -> (qkvT, bias_vecT).

            Emitted one bn ahead so the DRAM round trip and the DVE stream
            transposes hide under the previous bn's attention matmuls."""
            n = bn % NH
            if bn == NH:
                _load_x(1)
            scr = scr_handles[bn]
            qkvT = qkvp.tile([128, 3 * HW], F32R, tag="qkv", name=f"qkvT{bn}")
            _proj(bn, qkvT, 0, n * DH)  # q only

            qwh = qwhp.tile([128, 8 * 126], F32, tag="qwh", name=f"qwh{bn}")
            for qt in range(8):
                ps = ps_misc.tile([128, 256], F32, tag="misc", name=f"qwhp{bn}_{qt}")
                nc.tensor.matmul(
                    ps[:],
                    qkvT[:, qt * 128 : qt * 128 + 128],
                    rel_sb[:],
                    start=True,
                    stop=True,
                )
                nc.scalar.copy(qwh[:, qt * 126 : qt * 126 + 126], ps[:, :126])

            nc.sync.dma_start(
                scr.ap().rearrange("(a p) r -> p a r", p=128),
                qwh[:].rearrange("p (a r) -> p a r", r=126),
            )
            wv = wvp.tile([128, 256], F32, tag="wv", name=f"wv{bn}")
            hv = wvp.tile([128, 256], F32, tag="hv", name=f"hv{bn}")
            skew_eng = nc.sync if bn < 2 else nc.gpsimd
            for sg in range(4):
                skew_eng.dma_start(
                    wv[32 * sg : 32 * sg + 32, :].rearrange("p (a j) -> p a j", j=32),
                    bass.AP(scr, 31 + 32256 * sg, [[125, 32], [4032, 8], [1, 32]]),
                )
                skew_eng.dma_start(
                    hv[32 * sg : 32 * sg + 32, :].rearrange("p (a j) -> p a j", j=32),
                    bass.AP(scr, 94 + 32255 * sg, [[126, 32], [4028, 8], [1, 32]]),
                )

            # StreamTranspose cannot produce f32r (verifier requires the
            # producer to round), so transpose into an F32 staging tile and
            # round with one tensor_copy.
            b32 = biasp.tile([64, HW], F32, tag="bias32", name=f"bias32_{bn}")
            for src, row0 in ((wv, 0), (hv, 32)):
                for sg in range(4):
                    nc.vector.transpose(
                        b32[row0 : row0 + 32, 256 * sg : 256 * sg + 256],
                        src[32 * sg : 32 * sg + 32, :],
                    )
            bias_vecT = biasp.tile([64, HW], F32R, tag="biasv", name=f"biasv{bn}")
            nc.vector.tensor_copy(bias_vecT[:], b32[:])
            return qkvT, bias_vecT

        def stage_kv(bn, qkvT):
            n = bn % NH
            _proj(bn, qkvT, 1, 512 + n * DH)  # k
            _proj(bn, qkvT, 2, 1024 + n * DH)  # v
            vnat = vnatp.tile([128, HW], F32R, tag="vnat", name=f"vnat{bn}")
            for kc in range(8):
                ps = ps_misc.tile([128, 128], F32R, tag="misc", name=f"vtr{bn}_{kc}")
                nc.tensor.transpose(
                    ps[:], qkvT[:, 2 * HW + kc * 128 : 2 * HW + kc * 128 + 128], id_sb[:]
                )
                nc.vector.tensor_copy(vnat[:, kc * 128 : kc * 128 + 128], ps[:])
            return vnat

        def stage_attn(bn, qkvT, bias_vecT, vnat):
            b, n = bn // NH, bn % NH

            den_ps = [
                ps_den.tile([1, 512], F32, tag="den", name=f"den{bn}_{i}")
                for i in range(2)
            ]
            out_ps = [
                ps_out.tile([128, 512], F32, tag="out", name=f"outp{bn}_{i}")
                for i in range(2)
            ]

            def emit_st(kc):
                pT = ptp.tile([128, HW], F32R, tag="pt", name=f"pt{bn}_{kc}")
                for qc in range(2):
                    st = ps_st.tile([128, 512], F32, tag="st", name=f"st{bn}_{kc}_{qc}")
                    nc.tensor.matmul(
                        st[:],
                        qkvT[:, HW + kc * 128 : HW + kc * 128 + 128],
                        qkvT[:, qc * 512 : qc * 512 + 512],
                        start=True,
                        stop=False,
                    )
                    nc.tensor.matmul(
                        st[:],
                        sel_sb[0:64, kc * 128 : kc * 128 + 128],
                        bias_vecT[:, qc * 512 : qc * 512 + 512],
                        start=False,
                        stop=True,
                    )
                    nc.scalar.activation(
                        pT[:, qc * 512 : qc * 512 + 512], st[:], EXP
                    )
                return pT

            def emit_dp(kc, pT):
                for qc in range(2):
                    nc.tensor.matmul(
                        den_ps[qc][:],
                        ones_sb[:],
                        pT[:, qc * 512 : qc * 512 + 512],
                        start=(kc == 0),
                        stop=(kc == 7),
                    )
                    nc.tensor.matmul(
                        out_ps[qc][:],
                        vnat[:, kc * 128 : kc * 128 + 128],
                        pT[:, qc * 512 : qc * 512 + 512],
                        start=(kc == 0),
                        stop=(kc == 7),
                    )

            prev = None  # one kc behind: den/pv hide the exp latency
            for kc in range(8):
                pT = emit_st(kc)
                if prev is not None:
                    emit_dp(kc - 1, prev)
                prev = pT
            emit_dp(7, prev)

            outT = outp.tile([128, HW], F32, tag="outt", name=f"outT{bn}")
            den_sb = recp.tile([1, HW], F32, tag="densb", name=f"densb{bn}")
            for qc in range(2):
                nc.vector.tensor_copy(
                    outT[:, qc * 512 : qc * 512 + 512], out_ps[qc][:]
                )
                nc.scalar.copy(
                    den_sb[:, qc * 512 : qc * 512 + 512], den_ps[qc][:]
                )
            nc.sync.dma_start(outd[b, n], outT[:])
            nc.sync.dma_start(dend[b, n], den_sb[:])

        # software pipeline: bias chains emitted two bn ahead of attention
        n_bn = B_LOC * NH
        from collections import deque

        states = deque([stage_bias(0)])
        kvs = deque([stage_kv(0, states[0][0])])
        states.append(stage_bias(1))
        for bn in range(n_bn):
            if bn + 1 < n_bn:
                kvs.append(stage_kv(bn + 1, states[1][0]))
            if bn + 2 < n_bn:
                states.append(stage_bias(bn + 2))
            qkvT, bias_vecT = states.popleft()
            stage_attn(bn, qkvT, bias_vecT, kvs.popleft())


def _build():
    if "nc" in _CACHE:
        return _CACHE["nc"]
    nc = bacc.Bacc("TRN2", target_bir_lowering=False, debug=False, num_devices=N_CORES)
    xd = nc.dram_tensor("x_r", [B_LOC, C, HW], F32R, kind="ExternalInput").ap()
    wd = nc.dram_tensor("w_t", [C, 3 * NH * DH], F32R, kind="ExternalInput").ap()
    reld = nc.dram_tensor("rel_t", [128, 256], F32R, kind="ExternalInput").ap()
    seld = nc.dram_tensor("sel", [128, 1024], F32R, kind="ExternalInput").ap()
    onesd = nc.dram_tensor("ones", [128, 1], F32R, kind="ExternalInput").ap()
    identd = nc.dram_tensor("ident", [128, 128], F32R, kind="ExternalInput").ap()
    outd = nc.dram_tensor("out_r", [B_LOC, NH, DH, HW], F32, kind="ExternalOutput").ap()
    dend = nc.dram_tensor("den_r", [B_LOC, NH, 1, HW], F32, kind="ExternalOutput").ap()
    scr_handles = [
        nc.dram_tensor(f"scr{i}", [HW, 126], F32) for i in range(B_LOC * NH)
    ]
    with tile.TileContext(nc) as tc:
        _emit(tc, nc, xd, wd, reld, seld, onesd, identd, outd, dend, scr_handles)
    nc.compile()
    _CACHE["nc"] = nc
    return nc


def _in_maps(x, w_qkv, height_rel, width_rel):
    x = np.asarray(x, np.float32)
    w_qkv = np.asarray(w_qkv, np.float32)
    height_rel = np.asarray(height_rel, np.float32)
    width_rel = np.asarray(width_rel, np.float32)

    w_t = np.ascontiguousarray(w_qkv.T)  # [C, 1536]
    w_t[:, 512:1024] *= np.float32(SCALE)  # fold softmax scale into k
    rel_t = np.zeros((128, 256), np.float32)
    rel_t[:, 0:63] = width_rel.T
    rel_t[:, 63:126] = height_rel.T
    sel = _sel_const()
    ones = np.ones((128, 1), np.float32)
    ident = np.eye(128, dtype=np.float32)

    # pre-swizzle x's hw columns into device f-order
    xf = x.reshape(B, C, HW)[:, :, _Q_OF_F]

    shared = {
        "w_t": w_t,
        "rel_t": rel_t,
        "sel": sel,
        "ones": ones,
        "ident": ident,
    }
    maps = []
    for i in range(N_CORES):
        xm = xf[i * B_LOC : (i + 1) * B_LOC]
        maps.append({"x_r": np.ascontiguousarray(xm), **shared})
    return maps


def _assemble(results):
    out = np.empty((B, 3 * NH * DH // 3, H, W), np.float32)  # (16, 512, 32, 32)
    for i, r in enumerate(results):
        arr = r["out_r"] / r["den_r"]  # [B_LOC, NH, DH, HW] / [B_LOC, NH, 1, HW]
        arr = arr[..., _F_OF_Q]  # undo the device-side f-ordering of hw columns
        for b in range(B_LOC):
            # flat order of reference output = q*512 + n*128 + d
            out[i * B_LOC + b] = (
                arr[b].transpose(2, 0, 1).reshape(512, 32, 32)
            )
    return out


def run(x, w_qkv, height_rel, width_rel, **spmd_kwargs):
    nc = _build()
    maps = _in_maps(x, w_qkv, height_rel, width_rel)
    res = run_bass_kernel_spmd(nc, maps, core_ids=list(range(N_CORES)), **spmd_kwargs)
    return _assemble(res.results), res


def kernel(x, w_qkv, height_rel, width_rel):
    out, _ = run(x, w_qkv, height_rel, width_rel)
    return out


# revision 22
# speedup vs baseline: 1.2141x; 1.1232x over previous
"""BottleneckAttn TRN2 kernel.

Reference computation (per batch b, head n, fp32):
    qkv = w_qkv @ x_b                      # (1536, 1024), 1x1 conv
    q, k, v per head: (1024, 128) with hw = h*32 + w
    logits[q,k] = SCALE * (q . k) + qw[q, 31 + w2(k) - w(q)] + qh[q, 31 + h2(k) - h(q)]
        where qw[q,r] = q . width_rel[r], qh[q,r] = q . height_rel[r]
    out = softmax(logits) @ v              # (1024, 128)
    output[b] flat index = q*512 + n*128 + d  -> reshape (512, 32, 32)

Device strategy (SPMD, 8 cores, 2 batches/core):
  - All matmuls computed in the TRANSPOSED softmax layout ST[k, q] so the
    attention probabilities come out of the exp directly in the layout the
    PV matmul needs as its moving operand (no P transposes).
  - ST = k @ qT via PE (operands swapped); the relative-position bias is
    folded in as a second accumulating matmul with a constant 0/1 selection
    matrix lhsT (rows select the shifted width/height tables per PSUM
    partition).
  - The per-query shifted tables (skew gather) are built by a DRAM round
    trip: qw/qh computed in [q, r] layout on PE, stored to a DRAM scratch,
    re-loaded with an affine skewed access pattern (contiguous 32-element
    runs), then rotated into [table_row, q] layout with DVE 32x32 stream
    transposes.
  - Softmax denominators via an all-ones stationary matmul accumulated over
    k tiles into a single PSUM bank (partitions 0/32 for the two q halves).
  - EVERY matmul runs bf16: measured fastest issue rate (~216ns per
    512-col matmul vs 227ns f32r), but ONLY when the PE instruction
    stream is dtype-homogeneous — mixing fp16/f32r modes costs ~100ns
    per switch. The sel stationary is zero-padded to K=128: the 64-row
    (row_grp) PE configuration measured ~2x the issue interval.
  - den/PV matmuls for k-tile kc are emitted after the ST+bias matmuls of
    kc+1 so the scalar-engine exp has a full k-tile of slack to hide in.
  - x is pre-swizzled to the device's f-order hw columns on the host, so
    the projection matmuls read plain contiguous slices.
"""

import os
import sys

import numpy as np

for _p in ("/opt/trn_rl_repo", "/root/.axon_site/_ro/trn_rl_repo"):
    if os.path.isdir(_p) and _p not in sys.path:
        sys.path.append(_p)

import ml_dtypes

import concourse.bass as bass
import concourse.mybir as mybir
import concourse.tile as tile
from concourse import bacc
from concourse.bass_utils import run_bass_kernel_spmd

B, C, H, W = 16, 512, 32, 32
HW = H * W
NH, DH = 4, 128
SCALE = DH ** -0.5
N_CORES = 8
B_LOC = B // N_CORES

F32 = mybir.dt.float32
F32R = mybir.dt.float32r
BF16 = mybir.dt.bfloat16
EXP = mybir.ActivationFunctionType.Exp

_CACHE = {}

# f-order permutation: device hw column f = 256*s + 32*qt + j for q = 128*qt + 32*s + j
_QS = np.arange(1024)
_F_OF_Q = (256 * ((_QS % 128) // 32) + 32 * (_QS // 128) + (_QS % 32)).astype(np.int64)
_Q_OF_F = np.argsort(_F_OF_Q)


def _sel_const():
    """sel[j, kc*128 + p]: j<32 selects shifted width row w2(p), j in
    [32,64) selects shifted height row h2(p). Rows 64..127 are ZERO
    padding: a full-height K=128 stationary avoids the slow 64-row
    (row_grp) PE configuration measured at ~2x the issue interval."""
    sel = np.zeros((128, 8 * 128), np.float32)
    for kc in range(8):
        for p in range(128):
            f = 128 * kc + p
            sg, qt, j = f // 256, (f % 256) // 32, f % 32
            q = 128 * qt + 32 * sg + j
            sel[q % 32, kc * 128 + p] = 1.0
            sel[32 + q // 32, kc * 128 + p] = 1.0
    return sel  # [128, 1024]


def _emit(tc, nc, xd, wd, reld, seld, onesd, identd, outd, dend, scr_handles):
    from contextlib import ExitStack

    ctx = ExitStack()
    with ctx:
        const = ctx.enter_context(tc.tile_pool(name="const", bufs=1))
        xpool = ctx.enter_context(tc.tile_pool(name="x", bufs=B_LOC))
        qkvp = ctx.enter_context(tc.tile_pool(name="qkv", bufs=3))
        qwhp = ctx.enter_context(tc.tile_pool(name="qwh", bufs=3))
        wvp = ctx.enter_context(tc.tile_pool(name="wv", bufs=3))
        biasp = ctx.enter_context(tc.tile_pool(name="biasv", bufs=3))
        vnatp = ctx.enter_context(tc.tile_pool(name="vnat", bufs=2))
        ptp = ctx.enter_context(tc.tile_pool(name="pt", bufs=3))
        outp = ctx.enter_context(tc.tile_pool(name="outt", bufs=2))
        recp = ctx.enter_context(tc.tile_pool(name="recip", bufs=2))
        # PSUM budget (8 banks): st 2 + out 2 + den 2 + misc 2
        ps_st = ctx.enter_context(tc.tile_pool(name="psst", bufs=2, space="PSUM"))
        ps_out = ctx.enter_context(tc.tile_pool(name="psout", bufs=2, space="PSUM"))
        ps_den = ctx.enter_context(tc.tile_pool(name="psden", bufs=2, space="PSUM"))
        ps_misc = ctx.enter_context(tc.tile_pool(name="psmisc", bufs=2, space="PSUM"))

        # ---- constants / weights (spread startup DMA over 4 queues) ----
        x_sb = []
        qs = [nc.scalar, nc.sync, nc.gpsimd]

        def _load_x(b):
            xb = xpool.tile([128, 4 * HW], BF16, tag="x", name=f"x_sb{b}")
            for kc4 in range(4):
                for qc in range(2):
                    xeng = qs[(kc4 * 2 + qc) % 3]
                    xeng.dma_start(
                        xb[:, kc4 * HW + qc * 512 : kc4 * HW + qc * 512 + 512],
                        xd[b][kc4 * 128 : kc4 * 128 + 128, qc * 512 : qc * 512 + 512],
                    )
            x_sb.append(xb)

        _load_x(0)
        wt_sb = const.tile([128, 4 * 1536], BF16, name="wt_sb")
        # h3-outer: all q weights land first, then k, then v; alternate queues
        for h3 in range(3):
            for kc4 in range(4):
                weng = qs[(h3 * 4 + kc4 + 1) % 3]
                weng.dma_start(
                    wt_sb[:, kc4 * 1536 + h3 * 512 : kc4 * 1536 + h3 * 512 + 512],
                    wd[kc4 * 128 : kc4 * 128 + 128, h3 * 512 : h3 * 512 + 512],
                )
        rel_sb = const.tile([128, 256], BF16, name="rel_sb")
        nc.gpsimd.dma_start(rel_sb[:], reld)
        sel_sb = const.tile([128, 1024], BF16, name="sel_sb")
        nc.gpsimd.dma_start(sel_sb[:], seld)
        ones_sb = const.tile([128, 1], BF16, name="ones_sb")
        nc.gpsimd.dma_start(ones_sb[:], onesd)
        id_sb = const.tile([128, 128], BF16, name="id_sb")
        nc.gpsimd.dma_start(id_sb[:], identd)

        def _proj(bn, qkvT, t, col0):
            b = bn // NH
            for qc in range(2):
                ps = ps_misc.tile(
                    [128, 512], F32, tag="misc", name=f"proj{bn}_{t}_{qc}"
                )
                for kc4 in range(4):
                    nc.tensor.matmul(
                        ps[:],
                        wt_sb[:, kc4 * 1536 + col0 : kc4 * 1536 + col0 + 128],
                        x_sb[b][:, kc4 * HW + qc * 512 : kc4 * HW + qc * 512 + 512],
                        start=(kc4 == 0),
                        stop=(kc4 == 3),
                    )
                # q copies on ACT: they gate the qwh matmuls' weight loads,
                # and the DVE queue is busy with transposes/vnat right then.
                if t == 0:
                    nc.scalar.copy(
                        qkvT[:, t * HW + qc * 512 : t * HW + qc * 512 + 512], ps[:]
                    )
                else:
                    nc.vector.tensor_copy(
                        qkvT[:, t * HW + qc * 512 : t * HW + qc * 512 + 512], ps[:]
                    )

        def stage_bias(bn):
            """q projection + rel tables + skew round trip -> (qkvT, bias_vecT).

            Emitted one bn ahead so the DRAM round trip and the DVE stream
            transposes hide under the previous bn's attention matmuls."""
            n = bn % NH
            if bn == NH:
                _load_x(1)
            scr = scr_handles[bn]
            qkvT = qkvp.tile([128, 3 * HW], BF16, tag="qkv", name=f"qkvT{bn}")
            _proj(bn, qkvT, 0, n * DH)  # q only

            qwh = qwhp.tile([128, 8 * 126], BF16, tag="qwh", name=f"qwh{bn}")
            for qt in range(8):
                ps = ps_misc.tile([128, 256], F32, tag="misc", name=f"qwhp{bn}_{qt}")
                nc.tensor.matmul(
                    ps[:],
                    qkvT[:, qt * 128 : qt * 128 + 128],
                    rel_sb[:],
                    start=True,
                    stop=True,
                )
                nc.scalar.copy(qwh[:, qt * 126 : qt * 126 + 126], ps[:, :126])

            nc.sync.dma_start(
                scr.ap().rearrange("(a p) r -> p a r", p=128),
                qwh[:].rearrange("p (a r) -> p a r", r=126),
            )
            wv = wvp.tile([128, 256], BF16, tag="wv", name=f"wv{bn}")
            hv = wvp.tile([128, 256], BF16, tag="hv", name=f"hv{bn}")
            skew_eng = nc.sync if bn < 2 else nc.gpsimd
            for sg in range(4):
                skew_eng.dma_start(
                    wv[32 * sg : 32 * sg + 32, :].rearrange("p (a j) -> p a j", j=32),
                    bass.AP(scr, 31 + 32256 * sg, [[125, 32], [4032, 8], [1, 32]]),
                )
                skew_eng.dma_start(
                    hv[32 * sg : 32 * sg + 32, :].rearrange("p (a j) -> p a j", j=32),
                    bass.AP(scr, 94 + 32255 * sg, [[126, 32], [4028, 8], [1, 32]]),
                )

            # Rows 64..127 multiply zero sel rows but must hold finite
            # values (not stale NaN bit patterns), so duplicate the tables.
            bias_vecT = biasp.tile([128, HW], BF16, tag="biasv", name=f"biasv{bn}")
            for src, row0 in ((wv, 0), (hv, 32)):
                for sg in range(4):
                    nc.vector.transpose(
                        bias_vecT[row0 : row0 + 32, 256 * sg : 256 * sg + 256],
                        src[32 * sg : 32 * sg + 32, :],
                    )
            nc.vector.tensor_copy(bias_vecT[64:128, :], bias_vecT[0:64, :])
            return qkvT, bias_vecT

        def stage_kv(bn, qkvT):
            n = bn % NH
            _proj(bn, qkvT, 1, 512 + n * DH)  # k
            _proj(bn, qkvT, 2, 1024 + n * DH)  # v
            vnat = vnatp.tile([128, HW], BF16, tag="vnat", name=f"vnat{bn}")
            # batch 4 transposes per PSUM tile so one wide copy drains them:
            # fewer DVE instructions and fewer ldweights waits on the drain.
            for half in range(2):
                ps = ps_misc.tile([128, 512], BF16, tag="misc", name=f"vtr{bn}_{half}")
                for j in range(4):
                    kc = half * 4 + j
                    nc.tensor.transpose(
                        ps[:, j * 128 : j * 128 + 128],
                        qkvT[:, 2 * HW + kc * 128 : 2 * HW + kc * 128 + 128],
                        id_sb[:],
                    )
                nc.vector.tensor_copy(
                    vnat[:, half * 512 : half * 512 + 512], ps[:]
                )
            return vnat

        def stage_attn(bn, qkvT, bias_vecT, vnat):
            b, n = bn // NH, bn % NH

            den_ps = [
                ps_den.tile([1, 512], F32, tag="den", name=f"den{bn}_{i}")
                for i in range(2)
            ]
            out_ps = [
                ps_out.tile([128, 512], F32, tag="out", name=f"outp{bn}_{i}")
                for i in range(2)
            ]

            def emit_st(kc):
                pT = ptp.tile([128, HW], BF16, tag="pt", name=f"pt{bn}_{kc}")
                for qc in range(2):
                    st = ps_st.tile([128, 512], F32, tag="st", name=f"st{bn}_{kc}_{qc}")
                    nc.tensor.matmul(
                        st[:],
                        qkvT[:, HW + kc * 128 : HW + kc * 128 + 128],
                        qkvT[:, qc * 512 : qc * 512 + 512],
                        start=True,
                        stop=False,
                    )
                    nc.tensor.matmul(
                        st[:],
                        sel_sb[:, kc * 128 : kc * 128 + 128],
                        bias_vecT[:, qc * 512 : qc * 512 + 512],
                        start=False,
                        stop=True,
                    )
                    nc.scalar.activation(
                        pT[:, qc * 512 : qc * 512 + 512], st[:], EXP
                    )
                return pT

            def emit_dp(kc, pT):
                for qc in range(2):
                    nc.tensor.matmul(
                        den_ps[qc][:],
                        ones_sb[:],
                        pT[:, qc * 512 : qc * 512 + 512],
                        start=(kc == 0),
                        stop=(kc == 7),
                    )
                    nc.tensor.matmul(
                        out_ps[qc][:],
                        vnat[:, kc * 128 : kc * 128 + 128],
                        pT[:, qc * 512 : qc * 512 + 512],
                        start=(kc == 0),
                        stop=(kc == 7),
                    )

            last = bn == B_LOC * NH - 1
            prev = None  # one kc behind: den/pv hide the exp latency
            for kc in range(8):
                pT = emit_st(kc)
                if last:
                    # no lookahead on the final bn: shortens the drain tail
                    emit_dp(kc, pT)
                elif prev is not None:
                    emit_dp(kc - 1, prev)
                prev = pT
            if not last:
                emit_dp(7, prev)

            outT = outp.tile([128, HW], F32, tag="outt", name=f"outT{bn}")
            den_sb = recp.tile([1, HW], F32, tag="densb", name=f"densb{bn}")
            for qc in range(2):
                nc.vector.tensor_copy(
                    outT[:, qc * 512 : qc * 512 + 512], out_ps[qc][:]
                )
                nc.scalar.copy(
                    den_sb[:, qc * 512 : qc * 512 + 512], den_ps[qc][:]
                )
                # kick the output DMA per q-half so the last store overlaps
                nc.sync.dma_start(
                    outd[b, n][:, qc * 512 : qc * 512 + 512],
                    outT[:, qc * 512 : qc * 512 + 512],
                )
            nc.sync.dma_start(dend[b, n], den_sb[:])

        # software pipeline: bias chains emitted two bn ahead of attention
        n_bn = B_LOC * NH
        from collections import deque

        states = deque([stage_bias(0)])
        kvs = deque([stage_kv(0, states[0][0])])
        states.append(stage_bias(1))
        for bn in range(n_bn):
            if bn + 1 < n_bn:
                kvs.append(stage_kv(bn + 1, states[1][0]))
            if bn + 2 < n_bn:
                states.append(stage_bias(bn + 2))
            qkvT, bias_vecT = states.popleft()
            stage_attn(bn, qkvT, bias_vecT, kvs.popleft())


def _build():
    if "nc" in _CACHE:
        return _CACHE["nc"]
    nc = bacc.Bacc("TRN2", target_bir_lowering=False, debug=False, num_devices=N_CORES)
    xd = nc.dram_tensor("x_r", [B_LOC, C, HW], BF16, kind="ExternalInput").ap()
    wd = nc.dram_tensor("w_t", [C, 3 * NH * DH], BF16, kind="ExternalInput").ap()
    reld = nc.dram_tensor("rel_t", [128, 256], BF16, kind="ExternalInput").ap()
    seld = nc.dram_tensor("sel", [128, 1024], BF16, kind="ExternalInput").ap()
    onesd = nc.dram_tensor("ones", [128, 1], BF16, kind="ExternalInput").ap()
    identd = nc.dram_tensor("ident", [128, 128], BF16, kind="ExternalInput").ap()
    outd = nc.dram_tensor("out_r", [B_LOC, NH, DH, HW], F32, kind="ExternalOutput").ap()
    dend = nc.dram_tensor("den_r", [B_LOC, NH, 1, HW], F32, kind="ExternalOutput").ap()
    scr_handles = [
        nc.dram_tensor(f"scr{i}", [HW, 126], BF16) for i in range(B_LOC * NH)
    ]
    with tile.TileContext(nc) as tc:
        _emit(tc, nc, xd, wd, reld, seld, onesd, identd, outd, dend, scr_handles)
    nc.compile()
    _CACHE["nc"] = nc
    return nc


def _in_maps(x, w_qkv, height_rel, width_rel):
    x = np.asarray(x, np.float32)
    w_qkv = np.asarray(w_qkv, np.float32)
    height_rel = np.asarray(height_rel, np.float32)
    width_rel = np.asarray(width_rel, np.float32)

    w_t = np.ascontiguousarray(w_qkv.T)  # [C, 1536]
    w_t[:, 512:1024] *= np.float32(SCALE)  # fold softmax scale into k
    w_t = w_t.astype(ml_dtypes.bfloat16)
    rel_t = np.zeros((128, 256), np.float32)
    rel_t[:, 0:63] = width_rel.T
    rel_t[:, 63:126] = height_rel.T
    rel_t = rel_t.astype(ml_dtypes.bfloat16)
    sel = _sel_const().astype(ml_dtypes.bfloat16)
    ones = np.ones((128, 1), np.float32).astype(ml_dtypes.bfloat16)
    ident = np.eye(128, dtype=np.float32).astype(ml_dtypes.bfloat16)

    # pre-swizzle x's hw columns into device f-order
    xf = x.reshape(B, C, HW)[:, :, _Q_OF_F].astype(ml_dtypes.bfloat16)

    shared = {
        "w_t": w_t,
        "rel_t": rel_t,
        "sel": sel,
        "ones": ones,
        "ident": ident,
    }
    maps = []
    for i in range(N_CORES):
        xm = xf[i * B_LOC : (i + 1) * B_LOC]
        maps.append({"x_r": np.ascontiguousarray(xm), **shared})
    return maps


def _assemble(results):
    out = np.empty((B, 3 * NH * DH // 3, H, W), np.float32)  # (16, 512, 32, 32)
    for i, r in enumerate(results):
        arr = r["out_r"] / r["den_r"]  # [B_LOC, NH, DH, HW] / [B_LOC, NH, 1, HW]
        arr = arr[..., _F_OF_Q]  # undo the device-side f-ordering of hw columns
        for b in range(B_LOC):
            # flat order of reference output = q*512 + n*128 + d
            out[i * B_LOC + b] = (
                arr[b].transpose(2, 0, 1).reshape(512, 32, 32)
            )
    return out


def run(x, w_qkv, height_rel, width_rel, **spmd_kwargs):
    nc = _build()
    maps = _in_maps(x, w_qkv, height_rel, width_rel)
    res = run_bass_kernel_spmd(nc, maps, core_ids=list(range(N_CORES)), **spmd_kwargs)
    return _assemble(res.results), res


def kernel(x, w_qkv, height_rel, width_rel):
    out, _ = run(x, w_qkv, height_rel, width_rel)
    return out


# revision 28
# speedup vs baseline: 1.2771x; 1.0519x over previous
"""BottleneckAttn TRN2 kernel.

Reference computation (per batch b, head n, fp32):
    qkv = w_qkv @ x_b                      # (1536, 1024), 1x1 conv
    q, k, v per head: (1024, 128) with hw = h*32 + w
    logits[q,k] = SCALE * (q . k) + qw[q, 31 + w2(k) - w(q)] + qh[q, 31 + h2(k) - h(q)]
        where qw[q,r] = q . width_rel[r], qh[q,r] = q . height_rel[r]
    out = softmax(logits) @ v              # (1024, 128)
    output[b] flat index = q*512 + n*128 + d  -> reshape (512, 32, 32)

Device strategy (SPMD, 8 cores, 2 batches/core):
  - All matmuls computed in the TRANSPOSED softmax layout ST[k, q] so the
    attention probabilities come out of the exp directly in the layout the
    PV matmul needs as its moving operand (no P transposes).
  - ST = k @ qT via PE (operands swapped); the relative-position bias is
    folded in as a second accumulating matmul with a constant 0/1 selection
    matrix lhsT (rows select the shifted width/height tables per PSUM
    partition).
  - The per-query shifted tables (skew gather) are built by a DRAM round
    trip: qw/qh computed in [q, r] layout on PE, stored to a DRAM scratch,
    re-loaded with an affine skewed access pattern (contiguous 32-element
    runs), then rotated into [table_row, q] layout with DVE 32x32 stream
    transposes.
  - Softmax denominators via an all-ones stationary matmul accumulated over
    k tiles into a single PSUM bank (partitions 0/64 for the two q halves).
  - EVERY matmul runs bf16: measured fastest issue rate (~216ns per
    512-col matmul vs 227ns f32r), but ONLY when the PE instruction
    stream is dtype-homogeneous — mixing fp16/f32r modes costs ~100ns
    per switch. The sel stationary is zero-padded to K=128: the 64-row
    (row_grp) PE configuration measured ~2x the issue interval.
  - den/PV matmuls for k-tile kc are emitted after the ST+bias matmuls of
    kc+1 so the scalar-engine exp has a full k-tile of slack to hide in.
  - x is pre-swizzled to the device's f-order hw columns on the host, so
    the projection matmuls read plain contiguous slices.
"""

import os
import sys

import numpy as np

for _p in ("/opt/trn_rl_repo", "/root/.axon_site/_ro/trn_rl_repo"):
    if os.path.isdir(_p) and _p not in sys.path:
        sys.path.append(_p)

import ml_dtypes

import concourse.bass as bass
import concourse.mybir as mybir
import concourse.tile as tile
from concourse import bacc
from concourse.bass_utils import run_bass_kernel_spmd

B, C, H, W = 16, 512, 32, 32
HW = H * W
NH, DH = 4, 128
SCALE = DH ** -0.5
N_CORES = 8
B_LOC = B // N_CORES

F32 = mybir.dt.float32
F32R = mybir.dt.float32r
BF16 = mybir.dt.bfloat16
EXP = mybir.ActivationFunctionType.Exp

_CACHE = {}

# f-order permutation: device hw column f = 256*s + 32*qt + j for q = 128*qt + 32*s + j
_QS = np.arange(1024)
_F_OF_Q = (256 * ((_QS % 128) // 32) + 32 * (_QS // 128) + (_QS % 32)).astype(np.int64)
_Q_OF_F = np.argsort(_F_OF_Q)


def _sel_const():
    """sel[j, kc*128 + p]: j<32 selects shifted width row w2(p), j in
    [32,64) selects shifted height row h2(p). Rows 64..127 are ZERO
    padding: a full-height K=128 stationary avoids the slow 64-row
    (row_grp) PE configuration measured at ~2x the issue interval."""
    sel = np.zeros((128, 8 * 128), np.float32)
    for kc in range(8):
        for p in range(128):
            f = 128 * kc + p
            sg, qt, j = f // 256, (f % 256) // 32, f % 32
            q = 128 * qt + 32 * sg + j
            sel[q % 32, kc * 128 + p] = 1.0
            sel[32 + q // 32, kc * 128 + p] = 1.0
    return sel  # [128, 1024]


def _emit(tc, nc, xd, wd, reld, seld, onesd, identd, outd, dend, scr_handles):
    from contextlib import ExitStack

    ctx = ExitStack()
    with ctx:
        const = ctx.enter_context(tc.tile_pool(name="const", bufs=1))
        xpool = ctx.enter_context(tc.tile_pool(name="x", bufs=B_LOC))
        qkvp = ctx.enter_context(tc.tile_pool(name="qkv", bufs=3))
        qwhp = ctx.enter_context(tc.tile_pool(name="qwh", bufs=3))
        wvp = ctx.enter_context(tc.tile_pool(name="wv", bufs=3))
        biasp = ctx.enter_context(tc.tile_pool(name="biasv", bufs=3))
        vnatp = ctx.enter_context(tc.tile_pool(name="vnat", bufs=2))
        ptp = ctx.enter_context(tc.tile_pool(name="pt", bufs=3))
        outp = ctx.enter_context(tc.tile_pool(name="outt", bufs=2))
        recp = ctx.enter_context(tc.tile_pool(name="recip", bufs=2))
        # PSUM budget (8 banks): st 2 + out 2 + den 1 + misc 3
        ps_st = ctx.enter_context(tc.tile_pool(name="psst", bufs=2, space="PSUM"))
        ps_out = ctx.enter_context(tc.tile_pool(name="psout", bufs=2, space="PSUM"))
        ps_den = ctx.enter_context(tc.tile_pool(name="psden", bufs=1, space="PSUM"))
        ps_misc = ctx.enter_context(tc.tile_pool(name="psmisc", bufs=3, space="PSUM"))

        # ---- constants / weights (spread startup DMA over 4 queues) ----
        x_sb = []
        qs = [nc.scalar, nc.sync, nc.gpsimd]

        def _load_x(b):
            xb = xpool.tile([128, 4 * HW], BF16, tag="x", name=f"x_sb{b}")
            for kc4 in range(4):
                for qc in range(2):
                    xeng = qs[(kc4 * 2 + qc) % 3]
                    xeng.dma_start(
                        xb[:, kc4 * HW + qc * 512 : kc4 * HW + qc * 512 + 512],
                        xd[b][kc4 * 128 : kc4 * 128 + 128, qc * 512 : qc * 512 + 512],
                    )
            x_sb.append(xb)

        _load_x(0)
        wt_sb = const.tile([128, 4 * 1536], BF16, name="wt_sb")
        # h3-outer: all q weights land first, then k, then v; alternate queues
        for h3 in range(3):
            for kc4 in range(4):
                weng = qs[(h3 * 4 + kc4 + 1) % 3]
                weng.dma_start(
                    wt_sb[:, kc4 * 1536 + h3 * 512 : kc4 * 1536 + h3 * 512 + 512],
                    wd[kc4 * 128 : kc4 * 128 + 128, h3 * 512 : h3 * 512 + 512],
                )
        rel_sb = const.tile([128, 256], BF16, name="rel_sb")
        nc.gpsimd.dma_start(rel_sb[:], reld)
        sel_sb = const.tile([128, 1024], BF16, name="sel_sb")
        nc.gpsimd.dma_start(sel_sb[:], seld)
        ones_sb = const.tile([128, 1], BF16, name="ones_sb")
        nc.gpsimd.dma_start(ones_sb[:], onesd)
        id_sb = const.tile([128, 128], BF16, name="id_sb")
        nc.gpsimd.dma_start(id_sb[:], identd)

        def _proj(bn, qkvT, t, col0):
            b = bn // NH
            for qc in range(2):
                ps = ps_misc.tile(
                    [128, 512], F32, tag="misc", name=f"proj{bn}_{t}_{qc}"
                )
                for kc4 in range(4):
                    nc.tensor.matmul(
                        ps[:],
                        wt_sb[:, kc4 * 1536 + col0 : kc4 * 1536 + col0 + 128],
                        x_sb[b][:, kc4 * HW + qc * 512 : kc4 * HW + qc * 512 + 512],
                        start=(kc4 == 0),
                        stop=(kc4 == 3),
                    )
                # q copies on ACT: they gate the qwh matmuls' weight loads,
                # and the DVE queue is busy with transposes/vnat right then.
                if t == 0:
                    nc.scalar.copy(
                        qkvT[:, t * HW + qc * 512 : t * HW + qc * 512 + 512], ps[:]
                    )
                else:
                    nc.vector.tensor_copy(
                        qkvT[:, t * HW + qc * 512 : t * HW + qc * 512 + 512], ps[:]
                    )

        def stage_bias(bn):
            """q projection + rel tables + skew round trip -> (qkvT, bias_vecT).

            Emitted one bn ahead so the DRAM round trip and the DVE stream
            transposes hide under the previous bn's attention matmuls."""
            n = bn % NH
            if bn == NH:
                _load_x(1)
            scr = scr_handles[bn]
            qkvT = qkvp.tile([128, 3 * HW], BF16, tag="qkv", name=f"qkvT{bn}")
            _proj(bn, qkvT, 0, n * DH)  # q only

            qwh = qwhp.tile([128, 8 * 126], BF16, tag="qwh", name=f"qwh{bn}")
            for qt in range(8):
                ps = ps_misc.tile([128, 256], F32, tag="misc", name=f"qwhp{bn}_{qt}")
                nc.tensor.matmul(
                    ps[:],
                    qkvT[:, qt * 128 : qt * 128 + 128],
                    rel_sb[:],
                    start=True,
                    stop=True,
                )
                nc.scalar.copy(qwh[:, qt * 126 : qt * 126 + 126], ps[:, :126])

            nc.sync.dma_start(
                scr.ap().rearrange("(a p) r -> p a r", p=128),
                qwh[:].rearrange("p (a r) -> p a r", r=126),
            )
            wv = wvp.tile([128, 256], BF16, tag="wv", name=f"wv{bn}")
            hv = wvp.tile([128, 256], BF16, tag="hv", name=f"hv{bn}")
            skew_eng = nc.sync if bn < 2 else nc.gpsimd
            for sg in range(4):
                skew_eng.dma_start(
                    wv[32 * sg : 32 * sg + 32, :].rearrange("p (a j) -> p a j", j=32),
                    bass.AP(scr, 31 + 32256 * sg, [[125, 32], [4032, 8], [1, 32]]),
                )
                skew_eng.dma_start(
                    hv[32 * sg : 32 * sg + 32, :].rearrange("p (a j) -> p a j", j=32),
                    bass.AP(scr, 94 + 32255 * sg, [[126, 32], [4028, 8], [1, 32]]),
                )

            # Rows 64..127 multiply zero sel rows but must hold finite
            # values (not stale NaN bit patterns), so duplicate the tables.
            bias_vecT = biasp.tile([128, HW], BF16, tag="biasv", name=f"biasv{bn}")
            for src, row0 in ((wv, 0), (hv, 32)):
                for sg in range(4):
                    nc.vector.transpose(
                        bias_vecT[row0 : row0 + 32, 256 * sg : 256 * sg + 256],
                        src[32 * sg : 32 * sg + 32, :],
                    )
            nc.vector.tensor_copy(bias_vecT[64:128, :], bias_vecT[0:64, :])
            return qkvT, bias_vecT

        def stage_kv(bn, qkvT):
            n = bn % NH
            _proj(bn, qkvT, 1, 512 + n * DH)  # k
            _proj(bn, qkvT, 2, 1024 + n * DH)  # v
            vnat = vnatp.tile([128, HW], BF16, tag="vnat", name=f"vnat{bn}")
            # batch 4 transposes per PSUM tile so one wide copy drains them:
            # fewer DVE instructions and fewer ldweights waits on the drain.
            for half in range(2):
                ps = ps_misc.tile([128, 512], BF16, tag="misc", name=f"vtr{bn}_{half}")
                for j in range(4):
                    kc = half * 4 + j
                    nc.tensor.transpose(
                        ps[:, j * 128 : j * 128 + 128],
                        qkvT[:, 2 * HW + kc * 128 : 2 * HW + kc * 128 + 128],
                        id_sb[:],
                    )
                # drain on ACT: these copies gate the next bn's projection
                # PSUM slots, and the DVE queue is long right here.
                nc.scalar.copy(vnat[:, half * 512 : half * 512 + 512], ps[:])
            return vnat

        def stage_attn(bn, qkvT, bias_vecT, vnat):
            b, n = bn // NH, bn % NH

            # one PSUM bank for both q-half denominators: rows 0 and 64
            # (64 is a PE quadrant boundary, a legal output tile position)
            den_all = ps_den.tile([65, 512], F32, tag="den", name=f"den{bn}")
            den_ps = [den_all[0:1, :], den_all[64:65, :]]
            out_ps = [
                ps_out.tile([128, 512], F32, tag="out", name=f"outp{bn}_{i}")
                for i in range(2)
            ]

            def emit_st(kc):
                pT = ptp.tile([128, HW], BF16, tag="pt", name=f"pt{bn}_{kc}")
                for qc in range(2):
                    st = ps_st.tile([128, 512], F32, tag="st", name=f"st{bn}_{kc}_{qc}")
                    nc.tensor.matmul(
                        st[:],
                        qkvT[:, HW + kc * 128 : HW + kc * 128 + 128],
                        qkvT[:, qc * 512 : qc * 512 + 512],
                        start=True,
                        stop=False,
                    )
                    nc.tensor.matmul(
                        st[:],
                        sel_sb[:, kc * 128 : kc * 128 + 128],
                        bias_vecT[:, qc * 512 : qc * 512 + 512],
                        start=False,
                        stop=True,
                    )
                    nc.scalar.activation(
                        pT[:, qc * 512 : qc * 512 + 512], st[:], EXP
                    )
                return pT

            def emit_dp(kc, pT):
                for qc in range(2):
                    nc.tensor.matmul(
                        den_ps[qc],
                        ones_sb[:],
                        pT[:, qc * 512 : qc * 512 + 512],
                        start=(kc == 0),
                        stop=(kc == 7),
                    )
                    nc.tensor.matmul(
                        out_ps[qc][:],
                        vnat[:, kc * 128 : kc * 128 + 128],
                        pT[:, qc * 512 : qc * 512 + 512],
                        start=(kc == 0),
                        stop=(kc == 7),
                    )

            last = bn == B_LOC * NH - 1
            from collections import deque as _dq

            pend = _dq()  # two kc behind: den/pv fully hide the exp latency
            for kc in range(8):
                pT = emit_st(kc)
                if last:
                    # no lookahead on the final bn: shortens the drain tail
                    emit_dp(kc, pT)
                    continue
                pend.append((kc, pT))
                if len(pend) > 2:
                    emit_dp(*pend.popleft())
            while pend:
                emit_dp(*pend.popleft())

            outT = outp.tile([128, HW], F32, tag="outt", name=f"outT{bn}")
            den_sb = recp.tile([1, HW], F32, tag="densb", name=f"densb{bn}")
            for qc in range(2):
                nc.vector.tensor_copy(
                    outT[:, qc * 512 : qc * 512 + 512], out_ps[qc][:]
                )
                nc.scalar.copy(
                    den_sb[:, qc * 512 : qc * 512 + 512], den_ps[qc]
                )
                # kick the output DMA per q-half so the last store overlaps
                nc.sync.dma_start(
                    outd[b, n][:, qc * 512 : qc * 512 + 512],
                    outT[:, qc * 512 : qc * 512 + 512],
                )
            nc.sync.dma_start(dend[b, n], den_sb[:])

        # software pipeline: bias chains emitted two bn ahead of attention
        n_bn = B_LOC * NH
        from collections import deque

        states = deque([stage_bias(0)])
        kvs = deque([stage_kv(0, states[0][0])])
        states.append(stage_bias(1))
        for bn in range(n_bn):
            if bn + 1 < n_bn:
                kvs.append(stage_kv(bn + 1, states[1][0]))
            if bn + 2 < n_bn:
                states.append(stage_bias(bn + 2))
            qkvT, bias_vecT = states.popleft()
            stage_attn(bn, qkvT, bias_vecT, kvs.popleft())


def _build():
    if "nc" in _CACHE:
        return _CACHE["nc"]
    nc = bacc.Bacc("TRN2", target_bir_lowering=False, debug=False, num_devices=N_CORES)
    xd = nc.dram_tensor("x_r", [B_LOC, C, HW], BF16, kind="ExternalInput").ap()
    wd = nc.dram_tensor("w_t", [C, 3 * NH * DH], BF16, kind="ExternalInput").ap()
    reld = nc.dram_tensor("rel_t", [128, 256], BF16, kind="ExternalInput").ap()
    seld = nc.dram_tensor("sel", [128, 1024], BF16, kind="ExternalInput").ap()
    onesd = nc.dram_tensor("ones", [128, 1], BF16, kind="ExternalInput").ap()
    identd = nc.dram_tensor("ident", [128, 128], BF16, kind="ExternalInput").ap()
    outd = nc.dram_tensor("out_r", [B_LOC, NH, DH, HW], F32, kind="ExternalOutput").ap()
    dend = nc.dram_tensor("den_r", [B_LOC, NH, 1, HW], F32, kind="ExternalOutput").ap()
    scr_handles = [
        nc.dram_tensor(f"scr{i}", [HW, 126], BF16) for i in range(B_LOC * NH)
    ]
    with tile.TileContext(nc) as tc:
        _emit(tc, nc, xd, wd, reld, seld, onesd, identd, outd, dend, scr_handles)
    nc.compile()
    _CACHE["nc"] = nc
    return nc


def _in_maps(x, w_qkv, height_rel, width_rel):
    x = np.asarray(x, np.float32)
    w_qkv = np.asarray(w_qkv, np.float32)
    height_rel = np.asarray(height_rel, np.float32)
    width_rel = np.asarray(width_rel, np.float32)

    w_t = np.ascontiguousarray(w_qkv.T)  # [C, 1536]
    w_t[:, 512:1024] *= np.float32(SCALE)  # fold softmax scale into k
    w_t = w_t.astype(ml_dtypes.bfloat16)
    rel_t = np.zeros((128, 256), np.float32)
    rel_t[:, 0:63] = width_rel.T
    rel_t[:, 63:126] = height_rel.T
    rel_t = rel_t.astype(ml_dtypes.bfloat16)
    sel = _sel_const().astype(ml_dtypes.bfloat16)
    ones = np.ones((128, 1), np.float32).astype(ml_dtypes.bfloat16)
    ident = np.eye(128, dtype=np.float32).astype(ml_dtypes.bfloat16)

    # pre-swizzle x's hw columns into device f-order
    xf = x.reshape(B, C, HW)[:, :, _Q_OF_F].astype(ml_dtypes.bfloat16)

    shared = {
        "w_t": w_t,
        "rel_t": rel_t,
        "sel": sel,
        "ones": ones,
        "ident": ident,
    }
    maps = []
    for i in range(N_CORES):
        xm = xf[i * B_LOC : (i + 1) * B_LOC]
        maps.append({"x_r": np.ascontiguousarray(xm), **shared})
    return maps


def _assemble(results):
    out = np.empty((B, 3 * NH * DH // 3, H, W), np.float32)  # (16, 512, 32, 32)
    for i, r in enumerate(results):
        arr = r["out_r"] / r["den_r"]  # [B_LOC, NH, DH, HW] / [B_LOC, NH, 1, HW]
        arr = arr[..., _F_OF_Q]  # undo the device-side f-ordering of hw columns
        for b in range(B_LOC):
            # flat order of reference output = q*512 + n*128 + d
            out[i * B_LOC + b] = (
                arr[b].transpose(2, 0, 1).reshape(512, 32, 32)
            )
    return out


def run(x, w_qkv, height_rel, width_rel, **spmd_kwargs):
    nc = _build()
    maps = _in_maps(x, w_qkv, height_rel, width_rel)
    res = run_bass_kernel_spmd(nc, maps, core_ids=list(range(N_CORES)), **spmd_kwargs)
    return _assemble(res.results), res


def kernel(x, w_qkv, height_rel, width_rel):
    out, _ = run(x, w_qkv, height_rel, width_rel)
    return out


# revision 30
# speedup vs baseline: 1.2941x; 1.0133x over previous
"""BottleneckAttn TRN2 kernel.

Reference computation (per batch b, head n, fp32):
    qkv = w_qkv @ x_b                      # (1536, 1024), 1x1 conv
    q, k, v per head: (1024, 128) with hw = h*32 + w
    logits[q,k] = SCALE * (q . k) + qw[q, 31 + w2(k) - w(q)] + qh[q, 31 + h2(k) - h(q)]
        where qw[q,r] = q . width_rel[r], qh[q,r] = q . height_rel[r]
    out = softmax(logits) @ v              # (1024, 128)
    output[b] flat index = q*512 + n*128 + d  -> reshape (512, 32, 32)

Device strategy (SPMD, 8 cores, 2 batches/core):
  - All matmuls computed in the TRANSPOSED softmax layout ST[k, q] so the
    attention probabilities come out of the exp directly in the layout the
    PV matmul needs as its moving operand (no P transposes).
  - ST = k @ qT via PE (operands swapped); the relative-position bias is
    folded in as a second accumulating matmul with a constant 0/1 selection
    matrix lhsT (rows select the shifted width/height tables per PSUM
    partition).
  - The per-query shifted tables (skew gather) are built by a DRAM round
    trip: qw/qh computed in [q, r] layout on PE, stored to a DRAM scratch,
    re-loaded with an affine skewed access pattern (contiguous 32-element
    runs), then rotated into [table_row, q] layout with DVE 32x32 stream
    transposes.
  - Softmax denominators via an all-ones stationary matmul accumulated over
    k tiles into a single PSUM bank (partitions 0/64 for the two q halves).
  - EVERY matmul runs bf16: measured fastest issue rate (~216ns per
    512-col matmul vs 227ns f32r), but ONLY when the PE instruction
    stream is dtype-homogeneous — mixing fp16/f32r modes costs ~100ns
    per switch. The sel stationary is zero-padded to K=128: the 64-row
    (row_grp) PE configuration measured ~2x the issue interval.
  - den/PV matmuls for k-tile kc are emitted after the ST+bias matmuls of
    kc+1 so the scalar-engine exp has a full k-tile of slack to hide in.
  - x is pre-swizzled to the device's f-order hw columns on the host, so
    the projection matmuls read plain contiguous slices.
"""

import os
import sys

import numpy as np

for _p in ("/opt/trn_rl_repo", "/root/.axon_site/_ro/trn_rl_repo"):
    if os.path.isdir(_p) and _p not in sys.path:
        sys.path.append(_p)

import ml_dtypes

import concourse.bass as bass
import concourse.mybir as mybir
import concourse.tile as tile
from concourse import bacc
from concourse.bass_utils import run_bass_kernel_spmd

B, C, H, W = 16, 512, 32, 32
HW = H * W
NH, DH = 4, 128
SCALE = DH ** -0.5
N_CORES = 8
B_LOC = B // N_CORES

F32 = mybir.dt.float32
F32R = mybir.dt.float32r
BF16 = mybir.dt.bfloat16
EXP = mybir.ActivationFunctionType.Exp

_CACHE = {}

# f-order permutation: device hw column f = 256*s + 32*qt + j for q = 128*qt + 32*s + j
_QS = np.arange(1024)
_F_OF_Q = (256 * ((_QS % 128) // 32) + 32 * (_QS // 128) + (_QS % 32)).astype(np.int64)
_Q_OF_F = np.argsort(_F_OF_Q)


def _sel_const():
    """sel[j, kc*128 + p]: j<32 selects shifted width row w2(p), j in
    [32,64) selects shifted height row h2(p). Rows 64..127 are ZERO
    padding: a full-height K=128 stationary avoids the slow 64-row
    (row_grp) PE configuration measured at ~2x the issue interval."""
    sel = np.zeros((128, 8 * 128), np.float32)
    for kc in range(8):
        for p in range(128):
            f = 128 * kc + p
            sg, qt, j = f // 256, (f % 256) // 32, f % 32
            q = 128 * qt + 32 * sg + j
            sel[q % 32, kc * 128 + p] = 1.0
            sel[32 + q // 32, kc * 128 + p] = 1.0
    return sel  # [128, 1024]


def _emit(tc, nc, xd, wd, reld, seld, onesd, identd, outd, dend, scr_handles):
    from contextlib import ExitStack

    ctx = ExitStack()
    with ctx:
        const = ctx.enter_context(tc.tile_pool(name="const", bufs=1))
        xpool = ctx.enter_context(tc.tile_pool(name="x", bufs=B_LOC))
        qkvp = ctx.enter_context(tc.tile_pool(name="qkv", bufs=3))
        qwhp = ctx.enter_context(tc.tile_pool(name="qwh", bufs=3))
        wvp = ctx.enter_context(tc.tile_pool(name="wv", bufs=3))
        biasp = ctx.enter_context(tc.tile_pool(name="biasv", bufs=3))
        vnatp = ctx.enter_context(tc.tile_pool(name="vnat", bufs=2))
        ptp = ctx.enter_context(tc.tile_pool(name="pt", bufs=3))
        outp = ctx.enter_context(tc.tile_pool(name="outt", bufs=2))
        recp = ctx.enter_context(tc.tile_pool(name="recip", bufs=2))
        # PSUM budget (8 banks): st 2 + out 2 + den 1 + misc 3
        ps_st = ctx.enter_context(tc.tile_pool(name="psst", bufs=2, space="PSUM"))
        ps_out = ctx.enter_context(tc.tile_pool(name="psout", bufs=2, space="PSUM"))
        ps_den = ctx.enter_context(tc.tile_pool(name="psden", bufs=1, space="PSUM"))
        ps_misc = ctx.enter_context(tc.tile_pool(name="psmisc", bufs=3, space="PSUM"))

        # ---- constants / weights (spread startup DMA over 4 queues) ----
        x_sb = []
        qs = [nc.scalar, nc.sync, nc.gpsimd]

        def _load_x(b):
            xb = xpool.tile([128, 4 * HW], BF16, tag="x", name=f"x_sb{b}")
            for kc4 in range(4):
                for qc in range(2):
                    xeng = qs[(kc4 * 2 + qc) % 3]
                    xeng.dma_start(
                        xb[:, kc4 * HW + qc * 512 : kc4 * HW + qc * 512 + 512],
                        xd[b][kc4 * 128 : kc4 * 128 + 128, qc * 512 : qc * 512 + 512],
                    )
            x_sb.append(xb)

        wt_sb = const.tile([128, 4 * 1536], BF16, name="wt_sb")

        def _load_w(h3):
            for kc4 in range(4):
                weng = qs[(h3 * 4 + kc4 + 1) % 3]
                weng.dma_start(
                    wt_sb[:, kc4 * 1536 + h3 * 512 : kc4 * 1536 + h3 * 512 + 512],
                    wd[kc4 * 128 : kc4 * 128 + 128, h3 * 512 : h3 * 512 + 512],
                )

        # q weights first (the first projection needs them before the x
        # bulk), then x batch 0, then k/v weights.
        _load_w(0)
        _load_x(0)
        _load_w(1)
        _load_w(2)
        rel_sb = const.tile([128, 256], BF16, name="rel_sb")
        nc.gpsimd.dma_start(rel_sb[:], reld)
        sel_sb = const.tile([128, 1024], BF16, name="sel_sb")
        nc.gpsimd.dma_start(sel_sb[:], seld)
        ones_sb = const.tile([128, 1], BF16, name="ones_sb")
        nc.gpsimd.dma_start(ones_sb[:], onesd)
        id_sb = const.tile([128, 128], BF16, name="id_sb")
        nc.gpsimd.dma_start(id_sb[:], identd)

        def _proj(bn, qkvT, t, col0):
            b = bn // NH
            for qc in range(2):
                ps = ps_misc.tile(
                    [128, 512], F32, tag="misc", name=f"proj{bn}_{t}_{qc}"
                )
                for kc4 in range(4):
                    nc.tensor.matmul(
                        ps[:],
                        wt_sb[:, kc4 * 1536 + col0 : kc4 * 1536 + col0 + 128],
                        x_sb[b][:, kc4 * HW + qc * 512 : kc4 * HW + qc * 512 + 512],
                        start=(kc4 == 0),
                        stop=(kc4 == 3),
                    )
                # q copies on ACT: they gate the qwh matmuls' weight loads,
                # and the DVE queue is busy with transposes/vnat right then.
                if t == 0:
                    nc.scalar.copy(
                        qkvT[:, t * HW + qc * 512 : t * HW + qc * 512 + 512], ps[:]
                    )
                else:
                    nc.vector.tensor_copy(
                        qkvT[:, t * HW + qc * 512 : t * HW + qc * 512 + 512], ps[:]
                    )

        def stage_bias(bn):
            """q projection + rel tables + skew round trip -> (qkvT, bias_vecT).

            Emitted one bn ahead so the DRAM round trip and the DVE stream
            transposes hide under the previous bn's attention matmuls."""
            n = bn % NH
            if bn == NH:
                _load_x(1)
            scr = scr_handles[bn]
            qkvT = qkvp.tile([128, 3 * HW], BF16, tag="qkv", name=f"qkvT{bn}")
            _proj(bn, qkvT, 0, n * DH)  # q only

            qwh = qwhp.tile([128, 8 * 126], BF16, tag="qwh", name=f"qwh{bn}")
            for qt in range(8):
                ps = ps_misc.tile([128, 256], F32, tag="misc", name=f"qwhp{bn}_{qt}")
                nc.tensor.matmul(
                    ps[:],
                    qkvT[:, qt * 128 : qt * 128 + 128],
                    rel_sb[:],
                    start=True,
                    stop=True,
                )
                nc.vector.tensor_copy(qwh[:, qt * 126 : qt * 126 + 126], ps[:, :126])

            nc.sync.dma_start(
                scr.ap().rearrange("(a p) r -> p a r", p=128),
                qwh[:].rearrange("p (a r) -> p a r", r=126),
            )
            wv = wvp.tile([128, 256], BF16, tag="wv", name=f"wv{bn}")
            hv = wvp.tile([128, 256], BF16, tag="hv", name=f"hv{bn}")
            skew_eng = nc.sync if bn < 2 else nc.gpsimd
            for sg in range(4):
                skew_eng.dma_start(
                    wv[32 * sg : 32 * sg + 32, :].rearrange("p (a j) -> p a j", j=32),
                    bass.AP(scr, 31 + 32256 * sg, [[125, 32], [4032, 8], [1, 32]]),
                )
                skew_eng.dma_start(
                    hv[32 * sg : 32 * sg + 32, :].rearrange("p (a j) -> p a j", j=32),
                    bass.AP(scr, 94 + 32255 * sg, [[126, 32], [4028, 8], [1, 32]]),
                )

            # Rows 64..127 multiply zero sel rows but must hold finite
            # values (not stale NaN bit patterns), so duplicate the tables.
            bias_vecT = biasp.tile([128, HW], BF16, tag="biasv", name=f"biasv{bn}")
            for src, row0 in ((wv, 0), (hv, 32)):
                for sg in range(4):
                    nc.vector.transpose(
                        bias_vecT[row0 : row0 + 32, 256 * sg : 256 * sg + 256],
                        src[32 * sg : 32 * sg + 32, :],
                    )
            nc.vector.tensor_copy(bias_vecT[64:128, :], bias_vecT[0:64, :])
            return qkvT, bias_vecT

        def stage_kv(bn, qkvT):
            n = bn % NH
            _proj(bn, qkvT, 1, 512 + n * DH)  # k
            _proj(bn, qkvT, 2, 1024 + n * DH)  # v
            vnat = vnatp.tile([128, HW], BF16, tag="vnat", name=f"vnat{bn}")
            # batch 4 transposes per PSUM tile so one wide copy drains them:
            # fewer DVE instructions and fewer ldweights waits on the drain.
            for half in range(2):
                ps = ps_misc.tile([128, 512], BF16, tag="misc", name=f"vtr{bn}_{half}")
                for j in range(4):
                    kc = half * 4 + j
                    nc.tensor.transpose(
                        ps[:, j * 128 : j * 128 + 128],
                        qkvT[:, 2 * HW + kc * 128 : 2 * HW + kc * 128 + 128],
                        id_sb[:],
                    )
                # drain on ACT: these copies gate the next bn's projection
                # PSUM slots, and the DVE queue is long right here.
                nc.scalar.copy(vnat[:, half * 512 : half * 512 + 512], ps[:])
            return vnat

        def stage_attn(bn, qkvT, bias_vecT, vnat):
            b, n = bn // NH, bn % NH

            # one PSUM bank for both q-half denominators: rows 0 and 64
            # (64 is a PE quadrant boundary, a legal output tile position)
            den_all = ps_den.tile([65, 512], F32, tag="den", name=f"den{bn}")
            den_ps = [den_all[0:1, :], den_all[64:65, :]]
            out_ps = [
                ps_out.tile([128, 512], F32, tag="out", name=f"outp{bn}_{i}")
                for i in range(2)
            ]

            def emit_st(kc):
                pT = ptp.tile([128, HW], BF16, tag="pt", name=f"pt{bn}_{kc}")
                for qc in range(2):
                    st = ps_st.tile([128, 512], F32, tag="st", name=f"st{bn}_{kc}_{qc}")
                    nc.tensor.matmul(
                        st[:],
                        qkvT[:, HW + kc * 128 : HW + kc * 128 + 128],
                        qkvT[:, qc * 512 : qc * 512 + 512],
                        start=True,
                        stop=False,
                    )
                    nc.tensor.matmul(
                        st[:],
                        sel_sb[:, kc * 128 : kc * 128 + 128],
                        bias_vecT[:, qc * 512 : qc * 512 + 512],
                        start=False,
                        stop=True,
                    )
                    nc.scalar.activation(
                        pT[:, qc * 512 : qc * 512 + 512], st[:], EXP
                    )
                return pT

            def emit_dp(kc, pT):
                for qc in range(2):
                    nc.tensor.matmul(
                        den_ps[qc],
                        ones_sb[:],
                        pT[:, qc * 512 : qc * 512 + 512],
                        start=(kc == 0),
                        stop=(kc == 7),
                    )
                    nc.tensor.matmul(
                        out_ps[qc][:],
                        vnat[:, kc * 128 : kc * 128 + 128],
                        pT[:, qc * 512 : qc * 512 + 512],
                        start=(kc == 0),
                        stop=(kc == 7),
                    )

            last = bn == B_LOC * NH - 1
            from collections import deque as _dq

            pend = _dq()  # two kc behind: den/pv fully hide the exp latency
            for kc in range(8):
                pT = emit_st(kc)
                if last:
                    # no lookahead on the final bn: shortens the drain tail
                    emit_dp(kc, pT)
                    continue
                pend.append((kc, pT))
                if len(pend) > 2:
                    emit_dp(*pend.popleft())
            while pend:
                emit_dp(*pend.popleft())

            outT = outp.tile([128, HW], F32, tag="outt", name=f"outT{bn}")
            den_sb = recp.tile([1, HW], F32, tag="densb", name=f"densb{bn}")
            for qc in range(2):
                nc.vector.tensor_copy(
                    outT[:, qc * 512 : qc * 512 + 512], out_ps[qc][:]
                )
                nc.scalar.copy(
                    den_sb[:, qc * 512 : qc * 512 + 512], den_ps[qc]
                )
                # kick the output DMA per q-half so the last store overlaps
                nc.sync.dma_start(
                    outd[b, n][:, qc * 512 : qc * 512 + 512],
                    outT[:, qc * 512 : qc * 512 + 512],
                )
            nc.sync.dma_start(dend[b, n], den_sb[:])

        # software pipeline: bias chains emitted two bn ahead of attention
        n_bn = B_LOC * NH
        from collections import deque

        states = deque([stage_bias(0)])
        kvs = deque([stage_kv(0, states[0][0])])
        states.append(stage_bias(1))
        for bn in range(n_bn):
            if bn + 1 < n_bn:
                kvs.append(stage_kv(bn + 1, states[1][0]))
            if bn + 2 < n_bn:
                states.append(stage_bias(bn + 2))
            qkvT, bias_vecT = states.popleft()
            stage_attn(bn, qkvT, bias_vecT, kvs.popleft())


def _build():
    if "nc" in _CACHE:
        return _CACHE["nc"]
    nc = bacc.Bacc("TRN2", target_bir_lowering=False, debug=False, num_devices=N_CORES)
    xd = nc.dram_tensor("x_r", [B_LOC, C, HW], BF16, kind="ExternalInput").ap()
    wd = nc.dram_tensor("w_t", [C, 3 * NH * DH], BF16, kind="ExternalInput").ap()
    reld = nc.dram_tensor("rel_t", [128, 256], BF16, kind="ExternalInput").ap()
    seld = nc.dram_tensor("sel", [128, 1024], BF16, kind="ExternalInput").ap()
    onesd = nc.dram_tensor("ones", [128, 1], BF16, kind="ExternalInput").ap()
    identd = nc.dram_tensor("ident", [128, 128], BF16, kind="ExternalInput").ap()
    outd = nc.dram_tensor("out_r", [B_LOC, NH, DH, HW], F32, kind="ExternalOutput").ap()
    dend = nc.dram_tensor("den_r", [B_LOC, NH, 1, HW], F32, kind="ExternalOutput").ap()
    scr_handles = [
        nc.dram_tensor(f"scr{i}", [HW, 126], BF16) for i in range(B_LOC * NH)
    ]
    with tile.TileContext(nc) as tc:
        _emit(tc, nc, xd, wd, reld, seld, onesd, identd, outd, dend, scr_handles)
    nc.compile()
    _CACHE["nc"] = nc
    return nc


def _in_maps(x, w_qkv, height_rel, width_rel):
    x = np.asarray(x, np.float32)
    w_qkv = np.asarray(w_qkv, np.float32)
    height_rel = np.asarray(height_rel, np.float32)
    width_rel = np.asarray(width_rel, np.float32)

    w_t = np.ascontiguousarray(w_qkv.T)  # [C, 1536]
    w_t[:, 512:1024] *= np.float32(SCALE)  # fold softmax scale into k
    w_t = w_t.astype(ml_dtypes.bfloat16)
    rel_t = np.zeros((128, 256), np.float32)
    rel_t[:, 0:63] = width_rel.T
    rel_t[:, 63:126] = height_rel.T
    rel_t = rel_t.astype(ml_dtypes.bfloat16)
    sel = _sel_const().astype(ml_dtypes.bfloat16)
    ones = np.ones((128, 1), np.float32).astype(ml_dtypes.bfloat16)
    ident = np.eye(128, dtype=np.float32).astype(ml_dtypes.bfloat16)

    # pre-swizzle x's hw columns into device f-order
    xf = x.reshape(B, C, HW)[:, :, _Q_OF_F].astype(ml_dtypes.bfloat16)

    shared = {
        "w_t": w_t,
        "rel_t": rel_t,
        "sel": sel,
        "ones": ones,
        "ident": ident,
    }
    maps = []
    for i in range(N_CORES):
        xm = xf[i * B_LOC : (i + 1) * B_LOC]
        maps.append({"x_r": np.ascontiguousarray(xm), **shared})
    return maps


def _assemble(results):
    out = np.empty((B, 3 * NH * DH // 3, H, W), np.float32)  # (16, 512, 32, 32)
    for i, r in enumerate(results):
        arr = r["out_r"] / r["den_r"]  # [B_LOC, NH, DH, HW] / [B_LOC, NH, 1, HW]
        arr = arr[..., _F_OF_Q]  # undo the device-side f-ordering of hw columns
        for b in range(B_LOC):
            # flat order of reference output = q*512 + n*128 + d
            out[i * B_LOC + b] = (
                arr[b].transpose(2, 0, 1).reshape(512, 32, 32)
            )
    return out


def run(x, w_qkv, height_rel, width_rel, **spmd_kwargs):
    nc = _build()
    maps = _in_maps(x, w_qkv, height_rel, width_rel)
    res = run_bass_kernel_spmd(nc, maps, core_ids=list(range(N_CORES)), **spmd_kwargs)
    return _assemble(res.results), res


def kernel(x, w_qkv, height_rel, width_rel):
    out, _ = run(x, w_qkv, height_rel, width_rel)
    return out


# revision 34
# speedup vs baseline: 1.3374x; 1.0334x over previous
"""BottleneckAttn TRN2 kernel.

Reference computation (per batch b, head n, fp32):
    qkv = w_qkv @ x_b                      # (1536, 1024), 1x1 conv
    q, k, v per head: (1024, 128) with hw = h*32 + w
    logits[q,k] = SCALE * (q . k) + qw[q, 31 + w2(k) - w(q)] + qh[q, 31 + h2(k) - h(q)]
        where qw[q,r] = q . width_rel[r], qh[q,r] = q . height_rel[r]
    out = softmax(logits) @ v              # (1024, 128)
    output[b] flat index = q*512 + n*128 + d  -> reshape (512, 32, 32)

Device strategy (SPMD, 8 cores, 2 batches/core):
  - All matmuls computed in the TRANSPOSED softmax layout ST[k, q] so the
    attention probabilities come out of the exp directly in the layout the
    PV matmul needs as its moving operand (no P transposes).
  - ST = k @ qT via PE (operands swapped); the relative-position bias is
    folded in as a second accumulating matmul with a constant 0/1 selection
    matrix lhsT (rows select the shifted width/height tables per PSUM
    partition).
  - The per-query shifted tables (skew gather) are built by a DRAM round
    trip: qw/qh computed in [q, r] layout on PE, stored to a DRAM scratch,
    re-loaded with an affine skewed access pattern (contiguous 32-element
    runs), then rotated into [table_row, q] layout with DVE 32x32 stream
    transposes.
  - Softmax denominators via an all-ones stationary matmul accumulated over
    k tiles into a single PSUM bank (partitions 0/64 for the two q halves).
  - EVERY matmul runs bf16: measured fastest issue rate (~216ns per
    512-col matmul vs 227ns f32r), but ONLY when the PE instruction
    stream is dtype-homogeneous — mixing fp16/f32r modes costs ~100ns
    per switch. The sel stationary is zero-padded to K=128: the 64-row
    (row_grp) PE configuration measured ~2x the issue interval.
  - den/PV matmuls for k-tile kc are emitted after the ST+bias matmuls of
    kc+1 so the scalar-engine exp has a full k-tile of slack to hide in.
  - x is pre-swizzled to the device's f-order hw columns on the host, so
    the projection matmuls read plain contiguous slices.
"""

import os
import sys

import numpy as np

for _p in ("/opt/trn_rl_repo", "/root/.axon_site/_ro/trn_rl_repo"):
    if os.path.isdir(_p) and _p not in sys.path:
        sys.path.append(_p)

import ml_dtypes

import concourse.bass as bass
import concourse.mybir as mybir
import concourse.tile as tile
from concourse import bacc
from concourse.bass_utils import run_bass_kernel_spmd

B, C, H, W = 16, 512, 32, 32
HW = H * W
NH, DH = 4, 128
SCALE = DH ** -0.5
N_CORES = 8
B_LOC = B // N_CORES

F32 = mybir.dt.float32
F32R = mybir.dt.float32r
BF16 = mybir.dt.bfloat16
EXP = mybir.ActivationFunctionType.Exp

_CACHE = {}

# f-order permutation: device hw column f = 256*s + 32*qt + j for q = 128*qt + 32*s + j
_QS = np.arange(1024)
_F_OF_Q = (256 * ((_QS % 128) // 32) + 32 * (_QS // 128) + (_QS % 32)).astype(np.int64)
_Q_OF_F = np.argsort(_F_OF_Q)


def _sel_const():
    """sel[j, kc*128 + p]: j<32 selects shifted width row w2(p), j in
    [32,64) selects shifted height row h2(p). Rows 64..127 are ZERO
    padding: a full-height K=128 stationary avoids the slow 64-row
    (row_grp) PE configuration measured at ~2x the issue interval."""
    sel = np.zeros((128, 8 * 128), np.float32)
    for kc in range(8):
        for p in range(128):
            f = 128 * kc + p
            sg, qt, j = f // 256, (f % 256) // 32, f % 32
            q = 128 * qt + 32 * sg + j
            sel[q % 32, kc * 128 + p] = 1.0
            sel[32 + q // 32, kc * 128 + p] = 1.0
    return sel  # [128, 1024]


def _emit(tc, nc, xd, wd, reld, seld, onesd, identd, outd, dend, scr_handles):
    from contextlib import ExitStack

    ctx = ExitStack()
    with ctx:
        const = ctx.enter_context(tc.tile_pool(name="const", bufs=1))
        xpool = ctx.enter_context(tc.tile_pool(name="x", bufs=B_LOC))
        qkvp = ctx.enter_context(tc.tile_pool(name="qkv", bufs=3))
        qwhp = ctx.enter_context(tc.tile_pool(name="qwh", bufs=3))
        wvp = ctx.enter_context(tc.tile_pool(name="wv", bufs=3))
        biasp = ctx.enter_context(tc.tile_pool(name="biasv", bufs=3))
        vnatp = ctx.enter_context(tc.tile_pool(name="vnat", bufs=2))
        ptp = ctx.enter_context(tc.tile_pool(name="pt", bufs=10))
        outp = ctx.enter_context(tc.tile_pool(name="outt", bufs=2))
        recp = ctx.enter_context(tc.tile_pool(name="recip", bufs=2))
        # PSUM budget (8 banks): st 2 + out 2 + den 1 + misc 3
        ps_st = ctx.enter_context(tc.tile_pool(name="psst", bufs=2, space="PSUM"))
        ps_out = ctx.enter_context(tc.tile_pool(name="psout", bufs=2, space="PSUM"))
        ps_den = ctx.enter_context(tc.tile_pool(name="psden", bufs=1, space="PSUM"))
        ps_misc = ctx.enter_context(tc.tile_pool(name="psmisc", bufs=3, space="PSUM"))

        # ---- constants / weights (spread startup DMA over 4 queues) ----
        x_sb = []
        qs = [nc.scalar, nc.sync, nc.gpsimd]

        def _load_x_half(xb, b, qc):
            for kc4 in range(4):
                xeng = qs[(kc4 * 2 + qc) % 3]
                xeng.dma_start(
                    xb[:, kc4 * HW + qc * 512 : kc4 * HW + qc * 512 + 512],
                    xd[b][kc4 * 128 : kc4 * 128 + 128, qc * 512 : qc * 512 + 512],
                )

        def _load_x(b):
            xb = xpool.tile([128, 4 * HW], BF16, tag="x", name=f"x_sb{b}")
            _load_x_half(xb, b, 0)
            _load_x_half(xb, b, 1)
            x_sb.append(xb)

        wt_sb = const.tile([128, 4 * 1536], BF16, name="wt_sb")

        def _load_w(h3):
            for kc4 in range(4):
                weng = qs[(h3 * 4 + kc4 + 1) % 3]
                weng.dma_start(
                    wt_sb[:, kc4 * 1536 + h3 * 512 : kc4 * 1536 + h3 * 512 + 512],
                    wd[kc4 * 128 : kc4 * 128 + 128, h3 * 512 : h3 * 512 + 512],
                )

        # startup order follows first-use: q weights, x(qc0), k weights,
        # x(qc1), v weights.
        _load_w(0)
        xb0 = xpool.tile([128, 4 * HW], BF16, tag="x", name="x_sb0")
        _load_x_half(xb0, 0, 0)
        _load_w(1)
        _load_x_half(xb0, 0, 1)
        _load_w(2)
        x_sb.append(xb0)
        rel_sb = const.tile([128, 256], BF16, name="rel_sb")
        nc.gpsimd.dma_start(rel_sb[:], reld)
        sel_sb = const.tile([128, 1024], BF16, name="sel_sb")
        nc.gpsimd.dma_start(sel_sb[:], seld)
        ones_sb = const.tile([128, 1], BF16, name="ones_sb")
        nc.gpsimd.dma_start(ones_sb[:], onesd)
        id_sb = const.tile([128, 128], BF16, name="id_sb")
        nc.gpsimd.dma_start(id_sb[:], identd)

        def _proj(bn, qkvT, t, col0):
            b = bn // NH
            for qc in range(2):
                ps = ps_misc.tile(
                    [128, 512], F32, tag="misc", name=f"proj{bn}_{t}_{qc}"
                )
                for kc4 in range(4):
                    nc.tensor.matmul(
                        ps[:],
                        wt_sb[:, kc4 * 1536 + col0 : kc4 * 1536 + col0 + 128],
                        x_sb[b][:, kc4 * HW + qc * 512 : kc4 * HW + qc * 512 + 512],
                        start=(kc4 == 0),
                        stop=(kc4 == 3),
                    )
                # q copies on ACT: they gate the qwh matmuls' weight loads,
                # and the DVE queue is busy with transposes/vnat right then.
                if t == 0:
                    nc.scalar.copy(
                        qkvT[:, t * HW + qc * 512 : t * HW + qc * 512 + 512], ps[:]
                    )
                else:
                    nc.vector.tensor_copy(
                        qkvT[:, t * HW + qc * 512 : t * HW + qc * 512 + 512], ps[:]
                    )

        def stage_bias(bn):
            """q projection + rel tables + skew round trip -> (qkvT, bias_vecT).

            Emitted one bn ahead so the DRAM round trip and the DVE stream
            transposes hide under the previous bn's attention matmuls."""
            n = bn % NH
            if bn == NH:
                _load_x(1)
            scr = scr_handles[bn]
            qkvT = qkvp.tile([128, 3 * HW], BF16, tag="qkv", name=f"qkvT{bn}")
            _proj(bn, qkvT, 0, n * DH)  # q only

            qwh = qwhp.tile([128, 8 * 126], BF16, tag="qwh", name=f"qwh{bn}")
            for qt in range(8):
                ps = ps_misc.tile([128, 256], F32, tag="misc", name=f"qwhp{bn}_{qt}")
                nc.tensor.matmul(
                    ps[:],
                    qkvT[:, qt * 128 : qt * 128 + 128],
                    rel_sb[:],
                    start=True,
                    stop=True,
                )
                nc.vector.tensor_copy(qwh[:, qt * 126 : qt * 126 + 126], ps[:, :126])

            nc.sync.dma_start(
                scr.ap().rearrange("(a p) r -> p a r", p=128),
                qwh[:].rearrange("p (a r) -> p a r", r=126),
            )
            wv = wvp.tile([128, 256], BF16, tag="wv", name=f"wv{bn}")
            hv = wvp.tile([128, 256], BF16, tag="hv", name=f"hv{bn}")
            skew_eng = nc.sync if bn < 2 else nc.gpsimd
            for sg in range(4):
                skew_eng.dma_start(
                    wv[32 * sg : 32 * sg + 32, :].rearrange("p (a j) -> p a j", j=32),
                    bass.AP(scr, 31 + 32256 * sg, [[125, 32], [4032, 8], [1, 32]]),
                )
                skew_eng.dma_start(
                    hv[32 * sg : 32 * sg + 32, :].rearrange("p (a j) -> p a j", j=32),
                    bass.AP(scr, 94 + 32255 * sg, [[126, 32], [4028, 8], [1, 32]]),
                )

            # Rows 64..127 multiply zero sel rows but must hold finite
            # values (not stale NaN bit patterns), so duplicate the tables.
            bias_vecT = biasp.tile([128, HW], BF16, tag="biasv", name=f"biasv{bn}")
            for src, row0 in ((wv, 0), (hv, 32)):
                for sg in range(4):
                    nc.vector.transpose(
                        bias_vecT[row0 : row0 + 32, 256 * sg : 256 * sg + 256],
                        src[32 * sg : 32 * sg + 32, :],
                    )
            nc.vector.tensor_copy(bias_vecT[64:128, :], bias_vecT[0:64, :])
            return qkvT, bias_vecT

        def stage_kv(bn, qkvT):
            n = bn % NH
            _proj(bn, qkvT, 1, 512 + n * DH)  # k
            _proj(bn, qkvT, 2, 1024 + n * DH)  # v
            vnat = vnatp.tile([128, HW], BF16, tag="vnat", name=f"vnat{bn}")
            # batch 4 transposes per PSUM tile so one wide copy drains them:
            # fewer DVE instructions and fewer ldweights waits on the drain.
            for half in range(2):
                ps = ps_misc.tile([128, 512], BF16, tag="misc", name=f"vtr{bn}_{half}")
                for j in range(4):
                    kc = half * 4 + j
                    nc.tensor.transpose(
                        ps[:, j * 128 : j * 128 + 128],
                        qkvT[:, 2 * HW + kc * 128 : 2 * HW + kc * 128 + 128],
                        id_sb[:],
                    )
                # drain on ACT: these copies gate the next bn's projection
                # PSUM slots, and the DVE queue is long right here.
                nc.scalar.copy(vnat[:, half * 512 : half * 512 + 512], ps[:])
            return vnat

        def stage_attn(bn, qkvT, bias_vecT, vnat):
            b, n = bn // NH, bn % NH

            # one PSUM bank for both q-half denominators: rows 0 and 64
            # (64 is a PE quadrant boundary, a legal output tile position)
            den_all = ps_den.tile([65, 512], F32, tag="den", name=f"den{bn}")
            den_ps = [den_all[0:1, :], den_all[64:65, :]]
            out_ps = [
                ps_out.tile([128, 512], F32, tag="out", name=f"outp{bn}_{i}")
                for i in range(2)
            ]

            def emit_st(kc):
                pT = ptp.tile([128, HW], BF16, tag="pt", name=f"pt{bn}_{kc}")
                for qc in range(2):
                    st = ps_st.tile([128, 512], F32, tag="st", name=f"st{bn}_{kc}_{qc}")
                    nc.tensor.matmul(
                        st[:],
                        qkvT[:, HW + kc * 128 : HW + kc * 128 + 128],
                        qkvT[:, qc * 512 : qc * 512 + 512],
                        start=True,
                        stop=False,
                    )
                    nc.tensor.matmul(
                        st[:],
                        sel_sb[:, kc * 128 : kc * 128 + 128],
                        bias_vecT[:, qc * 512 : qc * 512 + 512],
                        start=False,
                        stop=True,
                    )
                    nc.scalar.activation(
                        pT[:, qc * 512 : qc * 512 + 512], st[:], EXP
                    )
                return pT

            def emit_pv(kc, pT):
                for qc in range(2):
                    nc.tensor.matmul(
                        out_ps[qc][:],
                        vnat[:, kc * 128 : kc * 128 + 128],
                        pT[:, qc * 512 : qc * 512 + 512],
                        start=(kc == 0),
                        stop=(kc == 7),
                    )

            def emit_den(kc, pT):
                for qc in range(2):
                    nc.tensor.matmul(
                        den_ps[qc],
                        ones_sb[:],
                        pT[:, qc * 512 : qc * 512 + 512],
                        start=(kc == 0),
                        stop=(kc == 7),
                    )

            def emit_dp(kc, pT):
                emit_den(kc, pT)
                emit_pv(kc, pT)

            # The M=1 den matmuls run in a different PE column-group config
            # than everything else; each entry/exit costs ~95ns. So the kc
            # loop interleaves only the (config-identical) PV matmuls at a
            # 3-tile lookahead, and ALL den matmuls run as one contiguous
            # block at the end of the head (the fp8 probability tiles stay
            # resident): 2 config switches per head instead of ~32.
            last = bn == B_LOC * NH - 1
            from collections import deque as _dq

            pend = _dq()
            pts = []
            for kc in range(8):
                pT = emit_st(kc)
                pts.append(pT)
                if last:
                    # no lookahead on the final bn: shortens the drain tail
                    emit_dp(kc, pT)
                    continue
                pend.append((kc, pT))
                if len(pend) > 3:
                    emit_pv(*pend.popleft())
            if not last:
                while pend:
                    emit_pv(*pend.popleft())
                for kc in range(8):
                    emit_den(kc, pts[kc])

            outT = outp.tile([128, HW], F32, tag="outt", name=f"outT{bn}")
            den_sb = recp.tile([1, HW], F32, tag="densb", name=f"densb{bn}")
            for qc in range(2):
                nc.vector.tensor_copy(
                    outT[:, qc * 512 : qc * 512 + 512], out_ps[qc][:]
                )
                nc.scalar.copy(
                    den_sb[:, qc * 512 : qc * 512 + 512], den_ps[qc]
                )
                # kick the output DMA per q-half so the last store overlaps
                nc.sync.dma_start(
                    outd[b, n][:, qc * 512 : qc * 512 + 512],
                    outT[:, qc * 512 : qc * 512 + 512],
                )
            nc.sync.dma_start(dend[b, n], den_sb[:])

        # software pipeline: bias chains emitted two bn ahead of attention
        n_bn = B_LOC * NH
        from collections import deque

        states = deque([stage_bias(0)])
        kvs = deque([stage_kv(0, states[0][0])])
        states.append(stage_bias(1))
        for bn in range(n_bn):
            if bn + 1 < n_bn:
                kvs.append(stage_kv(bn + 1, states[1][0]))
            if bn + 2 < n_bn:
                states.append(stage_bias(bn + 2))
            qkvT, bias_vecT = states.popleft()
            stage_attn(bn, qkvT, bias_vecT, kvs.popleft())


def _build():
    if "nc" in _CACHE:
        return _CACHE["nc"]
    nc = bacc.Bacc("TRN2", target_bir_lowering=False, debug=False, num_devices=N_CORES)
    xd = nc.dram_tensor("x_r", [B_LOC, C, HW], BF16, kind="ExternalInput").ap()
    wd = nc.dram_tensor("w_t", [C, 3 * NH * DH], BF16, kind="ExternalInput").ap()
    reld = nc.dram_tensor("rel_t", [128, 256], BF16, kind="ExternalInput").ap()
    seld = nc.dram_tensor("sel", [128, 1024], BF16, kind="ExternalInput").ap()
    onesd = nc.dram_tensor("ones", [128, 1], BF16, kind="ExternalInput").ap()
    identd = nc.dram_tensor("ident", [128, 128], BF16, kind="ExternalInput").ap()
    outd = nc.dram_tensor("out_r", [B_LOC, NH, DH, HW], F32, kind="ExternalOutput").ap()
    dend = nc.dram_tensor("den_r", [B_LOC, NH, 1, HW], F32, kind="ExternalOutput").ap()
    scr_handles = [
        nc.dram_tensor(f"scr{i}", [HW, 126], BF16) for i in range(B_LOC * NH)
    ]
    with tile.TileContext(nc) as tc:
        _emit(tc, nc, xd, wd, reld, seld, onesd, identd, outd, dend, scr_handles)
    nc.compile()
    _CACHE["nc"] = nc
    return nc


def _in_maps(x, w_qkv, height_rel, width_rel):
    x = np.asarray(x, np.float32)
    w_qkv = np.asarray(w_qkv, np.float32)
    height_rel = np.asarray(height_rel, np.float32)
    width_rel = np.asarray(width_rel, np.float32)

    w_t = np.ascontiguousarray(w_qkv.T)  # [C, 1536]
    w_t[:, 512:1024] *= np.float32(SCALE)  # fold softmax scale into k
    w_t = w_t.astype(ml_dtypes.bfloat16)
    rel_t = np.zeros((128, 256), np.float32)
    rel_t[:, 0:63] = width_rel.T
    rel_t[:, 63:126] = height_rel.T
    rel_t = rel_t.astype(ml_dtypes.bfloat16)
    sel = _sel_const().astype(ml_dtypes.bfloat16)
    ones = np.ones((128, 1), np.float32).astype(ml_dtypes.bfloat16)
    ident = np.eye(128, dtype=np.float32).astype(ml_dtypes.bfloat16)

    # pre-swizzle x's hw columns into device f-order
    xf = x.reshape(B, C, HW)[:, :, _Q_OF_F].astype(ml_dtypes.bfloat16)

    shared = {
        "w_t": w_t,
        "rel_t": rel_t,
        "sel": sel,
        "ones": ones,
        "ident": ident,
    }
    maps = []
    for i in range(N_CORES):
        xm = xf[i * B_LOC : (i + 1) * B_LOC]
        maps.append({"x_r": np.ascontiguousarray(xm), **shared})
    return maps


def _assemble(results):
    out = np.empty((B, 3 * NH * DH // 3, H, W), np.float32)  # (16, 512, 32, 32)
    for i, r in enumerate(results):
        arr = r["out_r"] / r["den_r"]  # [B_LOC, NH, DH, HW] / [B_LOC, NH, 1, HW]
        arr = arr[..., _F_OF_Q]  # undo the device-side f-ordering of hw columns
        for b in range(B_LOC):
            # flat order of reference output = q*512 + n*128 + d
            out[i * B_LOC + b] = (
                arr[b].transpose(2, 0, 1).reshape(512, 32, 32)
            )
    return out


def run(x, w_qkv, height_rel, width_rel, **spmd_kwargs):
    nc = _build()
    maps = _in_maps(x, w_qkv, height_rel, width_rel)
    res = run_bass_kernel_spmd(nc, maps, core_ids=list(range(N_CORES)), **spmd_kwargs)
    return _assemble(res.results), res


def kernel(x, w_qkv, height_rel, width_rel):
    out, _ = run(x, w_qkv, height_rel, width_rel)
    return out


# revision 38
# speedup vs baseline: 1.3748x; 1.0280x over previous
"""BottleneckAttn TRN2 kernel.

Reference computation (per batch b, head n, fp32):
    qkv = w_qkv @ x_b                      # (1536, 1024), 1x1 conv
    q, k, v per head: (1024, 128) with hw = h*32 + w
    logits[q,k] = SCALE * (q . k) + qw[q, 31 + w2(k) - w(q)] + qh[q, 31 + h2(k) - h(q)]
        where qw[q,r] = q . width_rel[r], qh[q,r] = q . height_rel[r]
    out = softmax(logits) @ v              # (1024, 128)
    output[b] flat index = q*512 + n*128 + d  -> reshape (512, 32, 32)

Device strategy (SPMD, 8 cores, 2 batches/core):
  - All matmuls computed in the TRANSPOSED softmax layout ST[k, q] so the
    attention probabilities come out of the exp directly in the layout the
    PV matmul needs as its moving operand (no P transposes).
  - ST = k @ qT via PE (operands swapped); the relative-position bias is
    folded in as a second accumulating matmul with a constant 0/1 selection
    matrix lhsT (rows select the shifted width/height tables per PSUM
    partition).
  - The per-query shifted tables (skew gather) are built by a DRAM round
    trip: qw/qh computed in [q, r] layout on PE, stored to a DRAM scratch,
    re-loaded with an affine skewed access pattern (contiguous 32-element
    runs), then rotated into [table_row, q] layout with DVE 32x32 stream
    transposes.
  - Softmax denominators via an all-ones stationary matmul accumulated over
    k tiles into a single PSUM bank (partitions 0/64 for the two q halves).
  - EVERY matmul runs bf16: measured fastest issue rate (~216ns per
    512-col matmul vs 227ns f32r), but ONLY when the PE instruction
    stream is dtype-homogeneous — mixing fp16/f32r modes costs ~100ns
    per switch. The sel stationary is zero-padded to K=128: the 64-row
    (row_grp) PE configuration measured ~2x the issue interval.
  - den/PV matmuls for k-tile kc are emitted after the ST+bias matmuls of
    kc+1 so the scalar-engine exp has a full k-tile of slack to hide in.
  - x is pre-swizzled to the device's f-order hw columns on the host, so
    the projection matmuls read plain contiguous slices.
"""

import os
import sys

import numpy as np

for _p in ("/opt/trn_rl_repo", "/root/.axon_site/_ro/trn_rl_repo"):
    if os.path.isdir(_p) and _p not in sys.path:
        sys.path.append(_p)

import ml_dtypes

import concourse.bass as bass
import concourse.mybir as mybir
import concourse.tile as tile
from concourse import bacc
from concourse.bass_utils import run_bass_kernel_spmd

B, C, H, W = 16, 512, 32, 32
HW = H * W
NH, DH = 4, 128
SCALE = DH ** -0.5
N_CORES = 8
B_LOC = B // N_CORES

F32 = mybir.dt.float32
F32R = mybir.dt.float32r
BF16 = mybir.dt.bfloat16
EXP = mybir.ActivationFunctionType.Exp

_CACHE = {}

# f-order permutation: device hw column f = 256*s + 32*qt + j for q = 128*qt + 32*s + j
_QS = np.arange(1024)
_F_OF_Q = (256 * ((_QS % 128) // 32) + 32 * (_QS // 128) + (_QS % 32)).astype(np.int64)
_Q_OF_F = np.argsort(_F_OF_Q)


def _sel_const():
    """sel[j, kc*128 + p]: j<32 selects shifted width row w2(p), j in
    [32,64) selects shifted height row h2(p). Rows 64..127 are ZERO
    padding: a full-height K=128 stationary avoids the slow 64-row
    (row_grp) PE configuration measured at ~2x the issue interval."""
    sel = np.zeros((128, 8 * 128), np.float32)
    for kc in range(8):
        for p in range(128):
            f = 128 * kc + p
            sg, qt, j = f // 256, (f % 256) // 32, f % 32
            q = 128 * qt + 32 * sg + j
            sel[q % 32, kc * 128 + p] = 1.0
            sel[32 + q // 32, kc * 128 + p] = 1.0
    return sel  # [128, 1024]


def _emit(tc, nc, xd, wd, reld, seld, onesd, identd, outd, dend, scr_handles):
    from contextlib import ExitStack

    ctx = ExitStack()
    with ctx:
        const = ctx.enter_context(tc.tile_pool(name="const", bufs=1))
        xpool = ctx.enter_context(tc.tile_pool(name="x", bufs=B_LOC))
        qkvp = ctx.enter_context(tc.tile_pool(name="qkv", bufs=3))
        qwhp = ctx.enter_context(tc.tile_pool(name="qwh", bufs=3))
        wvp = ctx.enter_context(tc.tile_pool(name="wv", bufs=3))
        biasp = ctx.enter_context(tc.tile_pool(name="biasv", bufs=3))
        vnatp = ctx.enter_context(tc.tile_pool(name="vnat", bufs=2))
        ptp = ctx.enter_context(tc.tile_pool(name="pt", bufs=10))
        outp = ctx.enter_context(tc.tile_pool(name="outt", bufs=2))
        recp = ctx.enter_context(tc.tile_pool(name="recip", bufs=2))
        # PSUM budget (8 banks): st 2 + out 2 + den 1 + misc 3
        ps_st = ctx.enter_context(tc.tile_pool(name="psst", bufs=2, space="PSUM"))
        ps_out = ctx.enter_context(tc.tile_pool(name="psout", bufs=2, space="PSUM"))
        ps_den = ctx.enter_context(tc.tile_pool(name="psden", bufs=1, space="PSUM"))
        ps_misc = ctx.enter_context(tc.tile_pool(name="psmisc", bufs=3, space="PSUM"))

        # ---- constants / weights (spread startup DMA over 4 queues) ----
        x_sb = []
        qs = [nc.scalar, nc.sync, nc.gpsimd]

        def _load_x_half(xb, b, qc):
            for kc4 in range(4):
                xeng = qs[(kc4 * 2 + qc) % 3]
                xeng.dma_start(
                    xb[:, kc4 * HW + qc * 512 : kc4 * HW + qc * 512 + 512],
                    xd[b][kc4 * 128 : kc4 * 128 + 128, qc * 512 : qc * 512 + 512],
                )

        def _load_x(b):
            xb = xpool.tile([128, 4 * HW], BF16, tag="x", name=f"x_sb{b}")
            _load_x_half(xb, b, 0)
            _load_x_half(xb, b, 1)
            x_sb.append(xb)

        wt_sb = const.tile([128, 4 * 1536], BF16, name="wt_sb")

        def _load_w(h3):
            for kc4 in range(4):
                weng = qs[(h3 * 4 + kc4 + 1) % 3]
                weng.dma_start(
                    wt_sb[:, kc4 * 1536 + h3 * 512 : kc4 * 1536 + h3 * 512 + 512],
                    wd[kc4 * 128 : kc4 * 128 + 128, h3 * 512 : h3 * 512 + 512],
                )

        # startup order follows first-use: q weights, x(qc0), k weights,
        # x(qc1), v weights.
        _load_w(0)
        xb0 = xpool.tile([128, 4 * HW], BF16, tag="x", name="x_sb0")
        _load_x_half(xb0, 0, 0)
        _load_w(1)
        _load_x_half(xb0, 0, 1)
        _load_w(2)
        x_sb.append(xb0)
        rel_sb = const.tile([128, 256], BF16, name="rel_sb")
        nc.gpsimd.dma_start(rel_sb[:], reld)
        sel_sb = const.tile([128, 1024], BF16, name="sel_sb")
        nc.gpsimd.dma_start(sel_sb[:], seld)
        ones_sb = const.tile([128, 1], BF16, name="ones_sb")
        nc.gpsimd.dma_start(ones_sb[:], onesd)
        id_sb = const.tile([128, 128], BF16, name="id_sb")
        nc.gpsimd.dma_start(id_sb[:], identd)

        def _proj(bn, qkvT, t, col0):
            b = bn // NH
            for qc in range(2):
                ps = ps_misc.tile(
                    [128, 512], F32, tag="misc", name=f"proj{bn}_{t}_{qc}"
                )
                for kc4 in range(4):
                    nc.tensor.matmul(
                        ps[:],
                        wt_sb[:, kc4 * 1536 + col0 : kc4 * 1536 + col0 + 128],
                        x_sb[b][:, kc4 * HW + qc * 512 : kc4 * HW + qc * 512 + 512],
                        start=(kc4 == 0),
                        stop=(kc4 == 3),
                    )
                # q and v copies on ACT: q gates the qwh matmuls' weight
                # loads and the DVE queue is long right then; v relieves
                # the DVE queue that drains the shared projection PSUM.
                if t in (0, 2):
                    nc.scalar.copy(
                        qkvT[:, t * HW + qc * 512 : t * HW + qc * 512 + 512], ps[:]
                    )
                else:
                    nc.vector.tensor_copy(
                        qkvT[:, t * HW + qc * 512 : t * HW + qc * 512 + 512], ps[:]
                    )

        def stage_bias(bn):
            """q projection + rel tables + skew round trip -> (qkvT, bias_vecT).

            Emitted one bn ahead so the DRAM round trip and the DVE stream
            transposes hide under the previous bn's attention matmuls."""
            n = bn % NH
            if bn == NH:
                _load_x(1)
            scr = scr_handles[bn]
            qkvT = qkvp.tile([128, 3 * HW], BF16, tag="qkv", name=f"qkvT{bn}")
            _proj(bn, qkvT, 0, n * DH)  # q only

            # two qt per PSUM tile, drained by one strided gpsimd copy:
            # halves the misc-slot churn and keeps DVE/ACT free (the misc
            # drains gate the next projection's PSUM slot).
            qwh = qwhp.tile([128, 8 * 126], BF16, tag="qwh", name=f"qwh{bn}")
            for qp in range(4):
                ps = ps_misc.tile([128, 512], F32, tag="misc", name=f"qwhp{bn}_{qp}")
                for j in range(2):
                    nc.tensor.matmul(
                        ps[:, j * 256 : j * 256 + 256],
                        qkvT[:, (qp * 2 + j) * 128 : (qp * 2 + j) * 128 + 128],
                        rel_sb[:],
                        start=True,
                        stop=True,
                    )
                nc.vector.tensor_copy(
                    qwh[:, qp * 252 : qp * 252 + 252].rearrange(
                        "p (j r) -> p j r", r=126
                    ),
                    ps[:].rearrange("p (j r) -> p j r", j=2)[:, :, 0:126],
                )

            nc.sync.dma_start(
                scr.ap().rearrange("(a p) r -> p a r", p=128),
                qwh[:].rearrange("p (a r) -> p a r", r=126),
            )
            wv = wvp.tile([128, 256], BF16, tag="wv", name=f"wv{bn}")
            hv = wvp.tile([128, 256], BF16, tag="hv", name=f"hv{bn}")
            skew_eng = nc.sync if bn < 2 else nc.gpsimd
            for sg in range(4):
                skew_eng.dma_start(
                    wv[32 * sg : 32 * sg + 32, :].rearrange("p (a j) -> p a j", j=32),
                    bass.AP(scr, 31 + 32256 * sg, [[125, 32], [4032, 8], [1, 32]]),
                )
                skew_eng.dma_start(
                    hv[32 * sg : 32 * sg + 32, :].rearrange("p (a j) -> p a j", j=32),
                    bass.AP(scr, 94 + 32255 * sg, [[126, 32], [4028, 8], [1, 32]]),
                )

            # Rows 64..127 multiply zero sel rows but must hold finite
            # values (not stale NaN bit patterns), so duplicate the tables.
            bias_vecT = biasp.tile([128, HW], BF16, tag="biasv", name=f"biasv{bn}")
            for src, row0 in ((wv, 0), (hv, 32)):
                for sg in range(4):
                    nc.vector.transpose(
                        bias_vecT[row0 : row0 + 32, 256 * sg : 256 * sg + 256],
                        src[32 * sg : 32 * sg + 32, :],
                    )
            nc.vector.tensor_copy(bias_vecT[64:128, :], bias_vecT[0:64, :])
            return qkvT, bias_vecT

        def stage_kv(bn, qkvT):
            n = bn % NH
            _proj(bn, qkvT, 1, 512 + n * DH)  # k
            _proj(bn, qkvT, 2, 1024 + n * DH)  # v
            vnat = vnatp.tile([128, HW], BF16, tag="vnat", name=f"vnat{bn}")
            # batch 4 transposes per PSUM tile so one wide copy drains them:
            # fewer DVE instructions and fewer ldweights waits on the drain.
            for half in range(2):
                ps = ps_misc.tile([128, 512], BF16, tag="misc", name=f"vtr{bn}_{half}")
                for j in range(4):
                    kc = half * 4 + j
                    nc.tensor.transpose(
                        ps[:, j * 128 : j * 128 + 128],
                        qkvT[:, 2 * HW + kc * 128 : 2 * HW + kc * 128 + 128],
                        id_sb[:],
                    )
                # drain on ACT: these copies gate the next bn's projection
                # PSUM slots, and the DVE queue is long right here.
                nc.scalar.copy(vnat[:, half * 512 : half * 512 + 512], ps[:])
            return vnat

        def stage_attn(bn, qkvT, bias_vecT, vnat):
            b, n = bn // NH, bn % NH

            # one PSUM bank for both q-half denominators: rows 0 and 64
            # (64 is a PE quadrant boundary, a legal output tile position)
            den_all = ps_den.tile([65, 512], F32, tag="den", name=f"den{bn}")
            den_ps = [den_all[0:1, :], den_all[64:65, :]]
            out_ps = [
                ps_out.tile([128, 512], F32, tag="out", name=f"outp{bn}_{i}")
                for i in range(2)
            ]

            def emit_st(kc):
                pT = ptp.tile([128, HW], BF16, tag="pt", name=f"pt{bn}_{kc}")
                for qc in range(2):
                    st = ps_st.tile([128, 512], F32, tag="st", name=f"st{bn}_{kc}_{qc}")
                    nc.tensor.matmul(
                        st[:],
                        qkvT[:, HW + kc * 128 : HW + kc * 128 + 128],
                        qkvT[:, qc * 512 : qc * 512 + 512],
                        start=True,
                        stop=False,
                    )
                    nc.tensor.matmul(
                        st[:],
                        sel_sb[:, kc * 128 : kc * 128 + 128],
                        bias_vecT[:, qc * 512 : qc * 512 + 512],
                        start=False,
                        stop=True,
                    )
                    nc.scalar.activation(
                        pT[:, qc * 512 : qc * 512 + 512], st[:], EXP
                    )
                return pT

            def emit_pv(kc, pT):
                for qc in range(2):
                    nc.tensor.matmul(
                        out_ps[qc][:],
                        vnat[:, kc * 128 : kc * 128 + 128],
                        pT[:, qc * 512 : qc * 512 + 512],
                        start=(kc == 0),
                        stop=(kc == 7),
                    )

            def emit_den(kc, pT):
                for qc in range(2):
                    nc.tensor.matmul(
                        den_ps[qc],
                        ones_sb[:],
                        pT[:, qc * 512 : qc * 512 + 512],
                        start=(kc == 0),
                        stop=(kc == 7),
                    )

            def emit_dp(kc, pT):
                emit_den(kc, pT)
                emit_pv(kc, pT)

            # The M=1 den matmuls run in a different PE column-group config
            # than everything else; each entry/exit costs ~95ns. So the kc
            # loop interleaves only the (config-identical) PV matmuls at a
            # 3-tile lookahead, and ALL den matmuls run as one contiguous
            # block at the end of the head (the fp8 probability tiles stay
            # resident): 2 config switches per head instead of ~32.
            last = bn == B_LOC * NH - 1
            from collections import deque as _dq

            pend = _dq()
            pts = []
            for kc in range(8):
                pT = emit_st(kc)
                pts.append(pT)
                if last:
                    # no lookahead on the final bn: shortens the drain tail
                    emit_dp(kc, pT)
                    continue
                pend.append((kc, pT))
                if len(pend) > 3:
                    emit_pv(*pend.popleft())
            if not last:
                while pend:
                    emit_pv(*pend.popleft())
                for kc in range(8):
                    emit_den(kc, pts[kc])

            outT = outp.tile([128, HW], F32, tag="outt", name=f"outT{bn}")
            den_sb = recp.tile([1, HW], F32, tag="densb", name=f"densb{bn}")
            for qc in range(2):
                nc.vector.tensor_copy(
                    outT[:, qc * 512 : qc * 512 + 512], out_ps[qc][:]
                )
                nc.scalar.copy(
                    den_sb[:, qc * 512 : qc * 512 + 512], den_ps[qc]
                )
                # kick the output DMA per q-half so the last store overlaps
                nc.sync.dma_start(
                    outd[b, n][:, qc * 512 : qc * 512 + 512],
                    outT[:, qc * 512 : qc * 512 + 512],
                )
            nc.sync.dma_start(dend[b, n], den_sb[:])

        # software pipeline: bias chains emitted two bn ahead of attention
        n_bn = B_LOC * NH
        from collections import deque

        states = deque([stage_bias(0)])
        kvs = deque([stage_kv(0, states[0][0])])
        states.append(stage_bias(1))
        for bn in range(n_bn):
            if bn + 1 < n_bn:
                kvs.append(stage_kv(bn + 1, states[1][0]))
            if bn + 2 < n_bn:
                states.append(stage_bias(bn + 2))
            qkvT, bias_vecT = states.popleft()
            stage_attn(bn, qkvT, bias_vecT, kvs.popleft())


def _build():
    if "nc" in _CACHE:
        return _CACHE["nc"]
    nc = bacc.Bacc("TRN2", target_bir_lowering=False, debug=False, num_devices=N_CORES)
    xd = nc.dram_tensor("x_r", [B_LOC, C, HW], BF16, kind="ExternalInput").ap()
    wd = nc.dram_tensor("w_t", [C, 3 * NH * DH], BF16, kind="ExternalInput").ap()
    reld = nc.dram_tensor("rel_t", [128, 256], BF16, kind="ExternalInput").ap()
    seld = nc.dram_tensor("sel", [128, 1024], BF16, kind="ExternalInput").ap()
    onesd = nc.dram_tensor("ones", [128, 1], BF16, kind="ExternalInput").ap()
    identd = nc.dram_tensor("ident", [128, 128], BF16, kind="ExternalInput").ap()
    outd = nc.dram_tensor("out_r", [B_LOC, NH, DH, HW], F32, kind="ExternalOutput").ap()
    dend = nc.dram_tensor("den_r", [B_LOC, NH, 1, HW], F32, kind="ExternalOutput").ap()
    scr_handles = [
        nc.dram_tensor(f"scr{i}", [HW, 126], BF16) for i in range(B_LOC * NH)
    ]
    with tile.TileContext(nc) as tc:
        _emit(tc, nc, xd, wd, reld, seld, onesd, identd, outd, dend, scr_handles)
    nc.compile()
    _CACHE["nc"] = nc
    return nc


def _in_maps(x, w_qkv, height_rel, width_rel):
    x = np.asarray(x, np.float32)
    w_qkv = np.asarray(w_qkv, np.float32)
    height_rel = np.asarray(height_rel, np.float32)
    width_rel = np.asarray(width_rel, np.float32)

    w_t = np.ascontiguousarray(w_qkv.T)  # [C, 1536]
    w_t[:, 512:1024] *= np.float32(SCALE)  # fold softmax scale into k
    w_t = w_t.astype(ml_dtypes.bfloat16)
    rel_t = np.zeros((128, 256), np.float32)
    rel_t[:, 0:63] = width_rel.T
    rel_t[:, 63:126] = height_rel.T
    rel_t = rel_t.astype(ml_dtypes.bfloat16)
    sel = _sel_const().astype(ml_dtypes.bfloat16)
    ones = np.ones((128, 1), np.float32).astype(ml_dtypes.bfloat16)
    ident = np.eye(128, dtype=np.float32).astype(ml_dtypes.bfloat16)

    # pre-swizzle x's hw columns into device f-order
    xf = x.reshape(B, C, HW)[:, :, _Q_OF_F].astype(ml_dtypes.bfloat16)

    shared = {
        "w_t": w_t,
        "rel_t": rel_t,
        "sel": sel,
        "ones": ones,
        "ident": ident,
    }
    maps = []
    for i in range(N_CORES):
        xm = xf[i * B_LOC : (i + 1) * B_LOC]
        maps.append({"x_r": np.ascontiguousarray(xm), **shared})
    return maps


def _assemble(results):
    out = np.empty((B, 3 * NH * DH // 3, H, W), np.float32)  # (16, 512, 32, 32)
    for i, r in enumerate(results):
        arr = r["out_r"] / r["den_r"]  # [B_LOC, NH, DH, HW] / [B_LOC, NH, 1, HW]
        arr = arr[..., _F_OF_Q]  # undo the device-side f-ordering of hw columns
        for b in range(B_LOC):
            # flat order of reference output = q*512 + n*128 + d
            out[i * B_LOC + b] = (
                arr[b].transpose(2, 0, 1).reshape(512, 32, 32)
            )
    return out


def run(x, w_qkv, height_rel, width_rel, **spmd_kwargs):
    nc = _build()
    maps = _in_maps(x, w_qkv, height_rel, width_rel)
    res = run_bass_kernel_spmd(nc, maps, core_ids=list(range(N_CORES)), **spmd_kwargs)
    return _assemble(res.results), res


def kernel(x, w_qkv, height_rel, width_rel):
    out, _ = run(x, w_qkv, height_rel, width_rel)
    return out
